# revision 47
# baseline (speedup 1.0000x reference)
"""Trainium2 Bass kernel for DSOAgent sampling (2-layer projected LSTM decode).

Math per step t (batch n, per core n=512):
  L0: gates = W_ih0 @ x_t + W_hh0 @ h0 + b0 ; c0' = sig(f)*c0 + sig(i)*tanh(g)
      h0' = (sig(o)*tanh(c0')) @ W_hr0.T
  L1: same with h0' as input -> h1'
  logits = h1' + prior[t];  p = softmax(logits)+eps (renorm ~1)
  ent[:,t] = -sum p*log(p);  lp[:,t] = log(p)[tokens[:,t]]

Sharding: pure data parallel, batch 4096 -> 8 cores x 512.

Device layout: feature-major [feat_part, batch_free] for the recurrence;
softmax done batch-major after a PE transpose.  Gate banks are ordered
(f, i, o, g) so one batched Sigmoid covers f,i,o.

Performance structure (what made it fast):
- Projection fusion on host: W_hh0@W_hr0, W_ih1@W_hr0, W_hh1@W_hr1 are
  pre-multiplied, so the recurrent state is the 128-dim pre-projection
  activation (hp) and no psum->SBUF state copies sit on the loop.  Step 0
  uses the unfused 64-dim form to consume h_init/c_init exactly.
- Biases never cost matmuls: layer 0's rides the ones-row in the xb
  moving chunk; layer 1's is applied as the per-partition ACT bias of
  per-gate full-width sigmoid/tanh ops (dropping the 4 K=1 bias matmuls
  per step cut ~1000 instructions and ~0.4ms of real-HW marginal time,
  and lets the cell update start as soon as the f-gate sigmoid lands).
- Each layer's gates use a 3-bank f/i/o psum pool + 1-bank g pool; the
  hr1/transpose/backlog scratch is time-multiplexed into the consumed
  g bank, so all 8 psum banks are productive and the sigma-critical banks
  recycle immediately after the Sigmoid reads them.
- The whole recurrence (matmuls, sigma via 3D strided APs, tanh, cell
  update, h_pre) is split into two independent 256-column batch halves
  that pipeline through PE->ACT->DVE; matmul groups are uniform
  half-width so psum accumulation-group tracking stays valid.
- Recurrence-chain ops are priority-boosted (tc.high_priority; L0 above
  L1) so the scheduler never wedges off-chain work into the serial loop.
- The softmax tail is computed per 32-step block WITHOUT materializing p
  or log(p): ent = lnZ - sum(x*e^x)/Z and lp_tok = ln(p_tok + eps) with
  p_tok = (sum_v onehot(tok)*e^x)/Z gathered by an is_equal mask and a
  3D tensor_reduce.  This replaces the per-(step,group) tiny
  scalar_tensor_tensor ops with a few wide DVE ops (drip-issued in
  8-step chunks across the next block) and shrinks the per-block ACT
  phase to three
  back-to-back ops (Exp, Ln(Z), Ln(p_tok)) under one exp/ln table phase
  (priority-boosted so no sigmoid wedges in and thrashes the table).
  bf16 e^x underflow at x<-87 reproduces the 1e-10 log clamp exactly.
- Full-width (512-col) matmuls and 4-step-batched xa/xb DMAs: the sim
  says half-width pipelines marginally better, but real-HW marginal
  per-exec time drops ~0.25ms from the halved PE instruction count
  (per-exec instruction fetch dominates the sim-vs-HW gap).
- bf16 matmul operands and cell state (forget-gate decay bounds drift;
  validated vs fp32), fp32 psum/logits so entropy/log-prob errors stay
  ~1e-3 relative (tolerance 2e-2).

Measurement: NTFF profiling is unavailable under axon here, so the "HW
exec time" is the marginal wall-clock per execution: back-to-back reps
on device-resident inputs (donated-output chaining serializes reps on
the cores), differenced between two rep counts to cancel the constant
dispatch round-trip.  Upper-bounds true device time (includes per-exec
NEFF launch overhead, ~0.8ms of the ~2ms).
"""

import os
from contextlib import ExitStack

import ml_dtypes
import numpy as np

import concourse.bass as bass
import concourse.tile as tile
from concourse import bacc, mybir
from concourse.bass_utils import run_bass_kernel_spmd
from concourse.tile_rust import add_dep_helper as _add_dep_raw


def add_dep_helper(frm, to, sync=True, reason=""):
    _add_dep_raw(getattr(frm, "ins", frm), getattr(to, "ins", to),
                 sync=sync, reason=reason)

F32 = mybir.dt.float32
BF16 = mybir.dt.bfloat16
AF = mybir.ActivationFunctionType
OP = mybir.AluOpType

T = 128          # decode steps
NB = 4096        # total batch
IN = 135         # input feature size
H = 128          # LSTM hidden
PJ = 64          # proj size / vocab
NCORES = 8
B = NB // NCORES  # per-core batch = 512
G = B // 128      # batch groups of 128 partitions = 4
KBLK = int(os.environ.get("K_KBLK", "32"))  # steps per softmax block
NBLK = T // KBLK
EPS = 1e-10

# batch-split factors per engine (env-settable for TimelineSim A/B runs);
# the recurrence is issued as independent per-slice chains that pipeline.
SPLIT_MM = int(os.environ.get("K_SPLIT_MM", "1"))
SPLIT_ACT = int(os.environ.get("K_SPLIT_ACT", "2"))
SPLIT_DVE = int(os.environ.get("K_SPLIT_DVE", "2"))
SPLIT_ACT1 = int(os.environ.get("K_SPLIT_ACT1", str(SPLIT_ACT)))
SPLIT_DVE1 = int(os.environ.get("K_SPLIT_DVE1", str(SPLIT_DVE)))
B1ACT = os.environ.get("K_B1ACT", "1") == "1"    # L1 bias via ACT bias operand
L0GATE = os.environ.get("K_L0GATE", "0") == "1"  # per-gate full-width L0 sigmoid


def _slices(n):
    w = 512 // n
    return [slice(i * w, (i + 1) * w) for i in range(n)]

# PyTorch gate order i,f,g,o; we reorder rows to (f, i, o, g) so the three
# sigmoid gates occupy adjacent psum banks.
def _gate_perm():
    i = np.arange(0, H)
    f = np.arange(H, 2 * H)
    g = np.arange(2 * H, 3 * H)
    o = np.arange(3 * H, 4 * H)
    return np.concatenate([f, i, o, g])


def _bf(x):
    return np.ascontiguousarray(x.astype(ml_dtypes.bfloat16))


def _f32(x):
    return np.ascontiguousarray(x.astype(np.float32))


def _length_priors_np():
    t = np.arange(T, dtype=np.float32)
    idx = np.arange(PJ)
    zero_mask = ((idx >= 0) & (idx < 32)).astype(np.float32)
    two_mask = ((idx >= 48) & (idx < 64)).astype(np.float32)
    pen_short = np.where(t < 64.0, -((64.0 - t) ** 2) / 16.0, 0.0).astype(np.float32)
    pen_long = np.where(t > 64.0, -((t - 64.0) ** 2) / 16.0, 0.0).astype(np.float32)
    return pen_short[:, None] * zero_mask[None, :] + pen_long[:, None] * two_mask[None, :]


def build_program(t_steps=T, kblk=KBLK):
    """Build and compile the single-core Bass program (same program runs on
    all 8 cores, SPMD over the batch)."""
    nblk = t_steps // kblk
    nc = bacc.Bacc(
        "TRN2",
        target_bir_lowering=False,
        debug=False,
        enable_asserts=False,
        num_devices=1,
    )

    # ---- DRAM I/O ----
    d_xa = nc.dram_tensor("xa", [t_steps, 128, B], BF16, kind="ExternalInput").ap()
    d_xb = nc.dram_tensor("xb", [t_steps, 7, B], BF16, kind="ExternalInput").ap()
    d_wa = nc.dram_tensor("wa", [128, 512], BF16, kind="ExternalInput").ap()
    d_wxb = nc.dram_tensor("wxb", [8, 512], BF16, kind="ExternalInput").ap()
    d_wh0f = nc.dram_tensor("wh0f", [128, 512], BF16, kind="ExternalInput").ap()
    d_wh0i = nc.dram_tensor("wh0i", [64, 512], BF16, kind="ExternalInput").ap()
    d_wA1 = nc.dram_tensor("wA1", [128, 512], BF16, kind="ExternalInput").ap()
    d_wB1 = nc.dram_tensor("wB1", [128, 512], BF16, kind="ExternalInput").ap()
    d_wh1i = nc.dram_tensor("wh1i", [64, 512], BF16, kind="ExternalInput").ap()
    d_b1r = nc.dram_tensor("b1r", [1, 512], BF16, kind="ExternalInput").ap()
    d_b1c = nc.dram_tensor("b1c", [128, 4], F32, kind="ExternalInput").ap()
    d_wr1 = nc.dram_tensor("wr1", [128, 64], BF16, kind="ExternalInput").ap()
    d_h0i = nc.dram_tensor("h0i", [64, B], BF16, kind="ExternalInput").ap()
    d_h1i = nc.dram_tensor("h1i", [64, B], BF16, kind="ExternalInput").ap()
    d_c0i = nc.dram_tensor("c0i", [128, B], BF16, kind="ExternalInput").ap()
    d_c1i = nc.dram_tensor("c1i", [128, B], BF16, kind="ExternalInput").ap()
    d_tok = nc.dram_tensor("tok", [128, G * t_steps], BF16, kind="ExternalInput").ap()
    d_pri = nc.dram_tensor("pri", [64, t_steps], F32, kind="ExternalInput").ap()
    d_iot = nc.dram_tensor("iot", [128, 64], BF16, kind="ExternalInput").ap()
    d_idn = nc.dram_tensor("idn", [64, 64], F32, kind="ExternalInput").ap()
    d_epb = nc.dram_tensor("epb", [128, 1], F32, kind="ExternalInput").ap()
    d_one = nc.dram_tensor("one", [1, B], BF16, kind="ExternalInput").ap()
    d_ent = nc.dram_tensor("ents", [128, G * t_steps], F32, kind="ExternalOutput").ap()
    d_lp = nc.dram_tensor("lps", [128, G * t_steps], F32, kind="ExternalOutput").ap()

    with tile.TileContext(nc) as tc, ExitStack() as ctx:
        _build_tile(ctx, tc, t_steps, kblk, nblk, dict(
            xa=d_xa, xb=d_xb, wa=d_wa, wxb=d_wxb, wh0f=d_wh0f, wh0i=d_wh0i,
            wA1=d_wA1, wB1=d_wB1, wh1i=d_wh1i, b1r=d_b1r, b1c=d_b1c, wr1=d_wr1, h0i=d_h0i, h1i=d_h1i, c0i=d_c0i, c1i=d_c1i, one=d_one,
            tok=d_tok, pri=d_pri, iot=d_iot, idn=d_idn, epb=d_epb, ent=d_ent, lp=d_lp,
        ))

    nc.compile()
    return nc


def _build_tile(ctx, tc, t_steps, kblk, nblk, io):
    nc = tc.nc

    cst = ctx.enter_context(tc.tile_pool(name="cst", bufs=1))
    st = ctx.enter_context(tc.tile_pool(name="st", bufs=1))
    wk = ctx.enter_context(tc.tile_pool(name="wk", bufs=3))
    wkx = ctx.enter_context(tc.tile_pool(name="wkx", bufs=3))
    pgL0f = ctx.enter_context(tc.tile_pool(name="pgL0f", bufs=1, space="PSUM"))
    pgL0g = ctx.enter_context(tc.tile_pool(name="pgL0g", bufs=1, space="PSUM"))
    pgL1f = ctx.enter_context(tc.tile_pool(name="pgL1f", bufs=1, space="PSUM"))
    pgL1g = ctx.enter_context(tc.tile_pool(name="pgL1g", bufs=1, space="PSUM"))

    def load_const(name, shape, dt):
        t_ = cst.tile(shape, dt, tag=name)
        nc.sync.dma_start(t_[:], io[name][:])
        return t_

    wa = load_const("wa", [128, 512], BF16)
    wxb = load_const("wxb", [8, 512], BF16)
    wh0f = load_const("wh0f", [128, 512], BF16)
    wh0i = load_const("wh0i", [64, 512], BF16)
    wA1 = load_const("wA1", [128, 512], BF16)
    wB1 = load_const("wB1", [128, 512], BF16)
    wh1i = load_const("wh1i", [64, 512], BF16)
    b1r = load_const("b1r", [1, 512], BF16)
    b1c = load_const("b1c", [128, 4], F32)
    one_t = load_const("one", [1, B], BF16)
    wr1 = load_const("wr1", [128, 64], BF16)
    tok = load_const("tok", [128, G * t_steps], BF16)
    pri = load_const("pri", [64, t_steps], F32)
    iot = load_const("iot", [128, 64], BF16)
    idn = load_const("idn", [64, 64], F32)
    epb = load_const("epb", [128, 1], F32)

    # persistent state (double-buffered across steps)
    # sxb: [xb(7); ones(1)] input chunk; hp0: layer-0 pre-projection state;
    # h1: [h1(64); ones(1)] layer-1 projected state; h0i/h1i initial h states
    # xa/xb stream in XB4-step batches (one DMA per tensor per XB4 steps)
    XB4 = int(os.environ.get("K_XB4", "4"))
    sxb = [st.tile([8, XB4 * B], BF16, tag=f"sxb_{k}", name=f"sxb_{k}")
           for k in range(2)]
    hp0s = [st.tile([128, B], BF16, tag=f"hp0s_{k}", name=f"hp0s_{k}") for k in range(2)]
    hp1s = [st.tile([128, B], BF16, tag=f"hp1s_{k}", name=f"hp1s_{k}") for k in range(2)]
    h0i = st.tile([64, B], BF16, tag="h0i", name="h0i")
    h1i = st.tile([64, B], BF16, tag="h1i", name="h1i")
    c0 = st.tile([128, B], BF16, tag="c0", name="c0")
    c1 = st.tile([128, B], BF16, tag="c1", name="c1")
    for k in range(2):
        for j in range(XB4):
            nc.sync.dma_start(sxb[k][7:8, j * B:(j + 1) * B], io["one"][:])
    nc.sync.dma_start(h0i[:], io["h0i"][:])
    nc.sync.dma_start(h1i[:], io["h1i"][:])
    nc.sync.dma_start(c0[:], io["c0i"][:])
    nc.sync.dma_start(c1[:], io["c1i"][:])

    # softmax block buffers.  Per block (16 steps x 4 groups x 64 vocab):
    #   e = exp(x); Z = sum_v e; ent = lnZ - sum_v(x*e)/Z
    #   lp_tok = ln(exp(x_tok - lnZ) + eps)  (x_tok via one-hot gather-reduce)
    backlog = [st.tile([128, kblk * 256], F32, tag=f"bl_{k}", name=f"bl_{k}") for k in range(2)]
    e_blk = st.tile([128, kblk * 256], BF16, tag="e_blk", name="e_blk")
    q_blk = st.tile([128, kblk * 256], BF16, tag="q_blk", name="q_blk")
    eq_blk = st.tile([128, kblk * 256], BF16, tag="eq_blk", name="eq_blk")
    ql_blk = q_blk  # shared scratch: s2 consumes q before ql writes (drip order)
    zs = st.tile([128, kblk * G], F32, tag="zs", name="zs")
    rz = st.tile([128, kblk * G], F32, tag="rz", name="rz")
    s2n = st.tile([128, kblk * G], F32, tag="s2n", name="s2n")
    ptk = st.tile([128, kblk * G], F32, tag="ptk", name="ptk")
    lnz = st.tile([128, kblk * G], F32, tag="lnz", name="lnz")
    pt2 = st.tile([128, kblk * G], F32, tag="pt2", name="pt2")
    se = st.tile([128, kblk * G], F32, tag="se", name="se")
    ent_o = st.tile([128, G * t_steps], F32, tag="ent_o", name="ent_o")
    lp_o = st.tile([128, G * t_steps], F32, tag="lp_o", name="lp_o")

    last_act = [None]   # last recurrence ACT op of current block
    deferred = []       # phase tail ops, drip-issued into the next block

    def act(*a, **k):
        op = nc.scalar.activation(*a, **k)
        last_act[0] = op
        return op

    def _sg4(tile_, off=0):
        """[128, kblk*256] -> [128, s, g, v] 4D view."""
        return tile_[:].rearrange("p (s g v) -> p s g v", g=G, v=64)

    def _sg(tile_):
        """[128, kblk*G] -> [128, s, g] view (s-major, matching reduces)."""
        return tile_[:].rearrange("p (s g) -> p s g", g=G)

    def _cols(out_tile, blk):
        """[128, s, g] strided view into out_tile's (g*T + blk*kblk + s) cols."""
        v = out_tile[:].rearrange("p (g t) -> p g t", t=t_steps)
        v = v[:, :, blk * kblk:(blk + 1) * kblk]
        return v.rearrange("p g s -> p s g")

    def _tok4(blk, s0, ns):
        """tokens for steps [blk*kblk+s0, +ns) as [128, ns, g, v(bcast)]."""
        v = tok[:].rearrange("p (g t) -> p g t", t=t_steps)
        v = v[:, :, blk * kblk + s0:blk * kblk + s0 + ns]
        v = v.rearrange("p g s -> p s g")
        return v.rearrange("p s (g o) -> p s g o", o=1).broadcast_to(
            [128, ns, G, 64])

    def softmax_phase(blk):
        """Emit the exp-table ACT group: Exp for block blk plus the two Ln
        ops finishing block blk-1 (all inputs pre-computed by the drip, so
        the three ops run back-to-back under one table phase)."""
        pbk = blk % 2
        with tc.high_priority(70000):
            exp_op = nc.scalar.activation(e_blk[:], backlog[pbk][:], AF.Exp)
            if os.environ.get("K_EXP_NODEP", "0") != "1":
                add_dep_helper(exp_op, last_act[0], sync=False,
                               reason="exp after recurrence ACT of block")
            if blk > 0:
                # lnZ of block blk-1 (zs ready from last block's drip)
                op = nc.scalar.activation(lnz[:], zs[:], AF.Ln)
                add_dep_helper(op, exp_op, sync=False, reason="lnz after exp")
                # lp(blk-1) = ln(p_tok + eps); p_tok = (sum eq*e)/Z from drip
                op2 = nc.scalar.activation(_cols(lp_o, blk - 1), _sg(pt2),
                                           AF.Ln, bias=epb[:])
                add_dep_helper(op2, op, sync=False, reason="lp after lnz")
        _emit_drip(blk)

    def _emit_drip(blk):
        """Deferred DVE work: ent combine for blk-1, then reduces for blk,
        split into s-halves so no single drip op overruns a step's DVE
        slack."""
        pbk = blk % 2
        nch = max(1, kblk // 8)                # 8-step drip chunks
        hw_, hc = kblk // nch, kblk // nch * 256
        hg = kblk // nch * G

        def h4(tile_, hs):
            return tile_[:, hs * hc:(hs + 1) * hc].rearrange(
                "p (s g v) -> p s g v", g=G, v=64)

        def hsg(tile_, hs):
            return tile_[:, hs * hg:(hs + 1) * hg].rearrange(
                "p (s g) -> p s g", g=G)

        if blk > 0:
            def _ent(blk=blk):
                # ent(blk-1) = lnz + (-s2)/Z  (lnz from this phase's ACT)
                nc.vector.tensor_tensor(se[:], s2n[:], rz[:], OP.mult)
                nc.vector.tensor_tensor(_cols(ent_o, blk - 1), _sg(lnz),
                                        _sg(se), OP.add)
            deferred.append(_ent)

        for hs in range(nch):
            def _z(hs=hs):
                nc.vector.tensor_reduce(hsg(zs, hs), h4(e_blk, hs),
                                        axis=mybir.AxisListType.X, op=OP.add)
                if hs == nch - 1:
                    nc.vector.reciprocal(rz[:], zs[:])
            deferred.append(_z)
        for hs in range(nch):
            def _q(hs=hs, pbk=pbk):
                nc.vector.tensor_tensor(
                    q_blk[:, hs * hc:(hs + 1) * hc],
                    backlog[pbk][:, hs * hc:(hs + 1) * hc],
                    e_blk[:, hs * hc:(hs + 1) * hc], OP.mult)
            deferred.append(_q)
        for hs in range(nch):
            def _s2(hs=hs):
                nc.vector.tensor_reduce(hsg(s2n, hs), h4(q_blk, hs),
                                        axis=mybir.AxisListType.X, op=OP.add,
                                        negate=True)
            deferred.append(_s2)
        for hs in range(nch):
            def _eq(hs=hs, blk=blk):
                iot4 = iot[:].rearrange("p (a b v) -> p a b v",
                                        a=1, b=1).broadcast_to(
                    [128, hw_, G, 64])
                tok4 = _tok4(blk, hs * hw_, hw_)
                nc.vector.tensor_tensor(h4(eq_blk, hs), iot4, tok4,
                                        OP.is_equal)
            deferred.append(_eq)
        for hs in range(nch):
            def _ql(hs=hs):
                nc.vector.tensor_tensor(
                    ql_blk[:, hs * hc:(hs + 1) * hc],
                    eq_blk[:, hs * hc:(hs + 1) * hc],
                    e_blk[:, hs * hc:(hs + 1) * hc], OP.mult)
            deferred.append(_ql)
        for hs in range(nch):
            def _ptk(hs=hs):
                nc.vector.tensor_reduce(hsg(ptk, hs), h4(ql_blk, hs),
                                        axis=mybir.AxisListType.X, op=OP.add)
            deferred.append(_ptk)

        def _pt2():
            nc.vector.tensor_tensor(pt2[:], ptk[:], rz[:], OP.mult)
        deferred.append(_pt2)

    def final_phase(blk):
        """Finish block blk's outputs at the end of the program."""
        op = nc.scalar.activation(lnz[:], zs[:], AF.Ln)
        add_dep_helper(op, last_act[0], sync=False, reason="final lnz")
        op2 = nc.scalar.activation(_cols(lp_o, blk), _sg(pt2), AF.Ln,
                                   bias=epb[:])
        add_dep_helper(op2, op, sync=False, reason="final lp")
        nc.vector.tensor_tensor(se[:], s2n[:], rz[:], OP.mult)
        nc.vector.tensor_tensor(_cols(ent_o, blk), _sg(lnz), _sg(se), OP.add)

    for t in range(t_steps):
        p_, pn = t % 2, (t + 1) % 2
        blk, s_in = t // kblk, t % kblk

        # input DMAs (batched: one DMA covers 4 steps)
        g4, s4 = t // XB4, t % XB4
        if s4 == 0:
            xa4_cur = wkx.tile([128, XB4 * B], BF16, tag="xa", name="xa")
            nc.sync.dma_start(
                xa4_cur[:].rearrange("p (t b) -> p t b", t=XB4),
                io["xa"][t:t + XB4].rearrange("t p b -> p t b"))
            nc.sync.dma_start(
                sxb[g4 % 2][0:7, :].rearrange("p (t b) -> p t b", t=XB4),
                io["xb"][t:t + XB4].rearrange("t p b -> p t b"))
        xa_tile, sxb_tile, xoff = xa4_cur, sxb[g4 % 2], s4 * B

        mm_slices = _slices(SPLIT_MM)
        act_slices = _slices(SPLIT_ACT)
        dve_slices = _slices(SPLIT_DVE)

        def gsl(g, sl):
            return slice(g * 512 + sl.start, g * 512 + sl.stop)

        # ---- layer 0 gates: psum banks (f, i, o) + (g) ----
        gf = pgL0f.tile([128, 1536], F32, tag="gL0f", name="gL0f")
        gg = pgL0g.tile([128, 512], F32, tag="gL0g", name="gL0g")
        with tc.high_priority(60000):
            for hsl in mm_slices:
                for m in range(4):
                    out = gf[:, m * 512:(m + 1) * 512] if m < 3 else gg[:]
                    msl = slice(m * 128, (m + 1) * 128)
                    nc.tensor.matmul(
                        out[:, hsl], wa[:, msl],
                        xa_tile[:, xoff + hsl.start:xoff + hsl.stop],
                        start=True, stop=False)
                    nc.tensor.matmul(
                        out[:, hsl], wxb[:, msl],
                        sxb_tile[:, xoff + hsl.start:xoff + hsl.stop],
                        start=False, stop=False)
                    if t == 0:
                        nc.tensor.matmul(out[:, hsl], wh0i[:, msl], h0i[:, hsl],
                                         start=False, stop=True)
                    else:
                        nc.tensor.matmul(out[:, hsl], wh0f[:, msl],
                                         hp0s[p_][:, hsl], start=False, stop=True)

        sfio = wk.tile([128, 1536], BF16, tag="sfio", name="sfio")
        with tc.high_priority(60000):
            gf3 = gf[:].rearrange("p (b n) -> p b n", n=512)
            sf3 = sfio[:].rearrange("p (b n) -> p b n", n=512)
            first_sig = None
            if L0GATE:
                for gi in range(3):
                    op = nc.scalar.activation(
                        sfio[:, gi * 512:(gi + 1) * 512],
                        gf[:, gi * 512:(gi + 1) * 512], AF.Sigmoid)
                    if first_sig is None:
                        first_sig = op
            else:
                for sl in act_slices:
                    op = nc.scalar.activation(sf3[:, :, sl], gf3[:, :, sl],
                                              AF.Sigmoid)
                    if first_sig is None:
                        first_sig = op
        # no hard exp->sigma gate: letting the scheduler interleave the next
        # block's recurrence through the ln/exp phase keeps PE warm; costs a
        # couple of extra ACT table loads per block (counted: 36 vs 24 total)
        # but nets faster overall.
        last_act[0] = first_sig
        tg = wk.tile([128, 512], BF16, tag="tg", name="tg")
        m0 = wk.tile([128, 512], BF16, tag="m0", name="m0")
        t1 = wk.tile([128, 512], BF16, tag="t1", name="t1")
        tc0 = wk.tile([128, 512], BF16, tag="tc0", name="tc0")
        hp0 = hp0s[pn]
        with tc.high_priority(60000):
            for sl in act_slices:
                act(tg[:, sl], gg[:, sl], AF.Tanh)
            for sl in dve_slices:
                nc.vector.tensor_tensor(
                    m0[:, sl], sfio[:, gsl(1, sl)], tg[:, sl], OP.mult)
                nc.vector.tensor_tensor(
                    t1[:, sl], sfio[:, gsl(0, sl)], c0[:, sl], OP.mult)
                nc.vector.tensor_tensor(c0[:, sl], m0[:, sl], t1[:, sl], OP.add)
            for sl in act_slices:
                act(tc0[:, sl], c0[:, sl], AF.Tanh)
            for sl in dve_slices:
                nc.vector.tensor_tensor(
                    hp0[:, sl], sfio[:, gsl(2, sl)], tc0[:, sl], OP.mult)

        # ---- layer 1 (input side fused with W_hr0) ----
        gf2 = pgL1f.tile([128, 1536], F32, tag="gL1f", name="gL1f")
        gg2 = pgL1g.tile([128, 512], F32, tag="gL1g", name="gL1g")
        with tc.high_priority(50000):
            for hsl in mm_slices:
                for m in ([3, 0, 1, 2] if os.environ.get("K_GFIRST", "0") == "1"
                          else range(4)):
                    out = gf2[:, m * 512:(m + 1) * 512] if m < 3 else gg2[:]
                    msl = slice(m * 128, (m + 1) * 128)
                    if not B1ACT:
                        nc.tensor.matmul(out[:, hsl], b1r[:, msl], one_t[:, hsl],
                                         start=True, stop=False)
                    if t == 0:
                        nc.tensor.matmul(out[:, hsl], wh1i[:, msl], h1i[:, hsl],
                                         start=B1ACT, stop=False)
                    else:
                        nc.tensor.matmul(out[:, hsl], wB1[:, msl],
                                         hp1s[p_][:, hsl], start=B1ACT, stop=False)
                    nc.tensor.matmul(out[:, hsl], wA1[:, msl], hp0[:, hsl],
                                     start=False, stop=True)

        sfio1 = wk.tile([128, 1536], BF16, tag="sfio1", name="sfio1")
        with tc.high_priority(50000):
            gf23 = gf2[:].rearrange("p (b n) -> p b n", n=512)
            sf13 = sfio1[:].rearrange("p (b n) -> p b n", n=512)
            tg1 = wk.tile([128, 512], BF16, tag="tg1", name="tg1")
            m1 = wk.tile([128, 512], BF16, tag="m1", name="m1")
            t11 = wk.tile([128, 512], BF16, tag="t11", name="t11")
            tc1 = wk.tile([128, 512], BF16, tag="tc1", name="tc1")
            hp1 = hp1s[pn]
            if B1ACT:
                # per-gate full-width sigmoids so each takes its own b1 bias;
                # tanh(g) takes the g-gate bias the same way
                for gi in range(3):
                    act(sfio1[:, gi * 512:(gi + 1) * 512],
                        gf2[:, gi * 512:(gi + 1) * 512], AF.Sigmoid,
                        bias=b1c[:, gi:gi + 1])
                act(tg1[:], gg2[:], AF.Tanh, bias=b1c[:, 3:4])
            else:
                for sl in _slices(SPLIT_ACT1):
                    if os.environ.get("K_GFIRST", "0") == "1":
                        act(tg1[:, sl], gg2[:, sl], AF.Tanh)
                        act(sf13[:, :, sl], gf23[:, :, sl], AF.Sigmoid)
                    else:
                        act(sf13[:, :, sl], gf23[:, :, sl], AF.Sigmoid)
                        act(tg1[:, sl], gg2[:, sl], AF.Tanh)
            for sl in _slices(SPLIT_DVE1):
                nc.vector.tensor_tensor(
                    m1[:, sl], sfio1[:, gsl(1, sl)], tg1[:, sl], OP.mult)
                nc.vector.tensor_tensor(
                    t11[:, sl], sfio1[:, gsl(0, sl)], c1[:, sl], OP.mult)
                nc.vector.tensor_tensor(c1[:, sl], m1[:, sl], t11[:, sl], OP.add)
            for sl in _slices(SPLIT_ACT1):
                act(tc1[:, sl], c1[:, sl], AF.Tanh)
            for sl in _slices(SPLIT_DVE1):
                nc.vector.tensor_tensor(
                    hp1[:, sl], sfio1[:, gsl(2, sl)], tc1[:, sl], OP.mult)

        aps_ = gg2[0:64, :]
        nc.tensor.matmul(aps_, wr1[:], hp1[:], start=True, stop=True,
                         skip_group_check=True)

        # logits = h1' + prior[t]  (f32, feature-major)
        lgt = wk.tile([64, 512], F32, tag="lgt", name="lgt")
        nc.vector.tensor_scalar(lgt[:], aps_, pri[:, t:t + 1], None, OP.add)

        # transpose to batch-major [128, 4*64] and copy to backlog
        bps = gg2[0:128, 0:256]
        for g in range(G):
            nc.tensor.matmul(
                bps[:, g * 64:(g + 1) * 64], lgt[:, g * 128:(g + 1) * 128], idn[:],
                is_transpose=True, skip_group_check=True)
        nc.vector.tensor_copy(backlog[blk % 2][:, s_in * 256:(s_in + 1) * 256], bps)

        # drip-issue deferred phase-tail ops (after the step body so they
        # rank below this step's tail ops in the scheduler's tie-breaks)
        if deferred:
            deferred.pop(0)()

        if s_in == kblk - 1:
            softmax_phase(blk)

    # final block: drain deferred, then finish its ent/lp outputs
    while deferred:
        deferred.pop(0)()
    final_phase(nblk - 1)

    nc.sync.dma_start(io["ent"][:], ent_o[:])
    nc.sync.dma_start(io["lp"][:], lp_o[:])


# ---------------------------------------------------------------------------
# host side
# ---------------------------------------------------------------------------

_CACHE = {}


def _get_program():
    if "nc" not in _CACHE:
        _CACHE["nc"] = build_program()
    return _CACHE["nc"]


class _PjrtRunner:
    """Executes the compiled Bass module on the 8 NeuronCores via PJRT with
    the input staging (host->device transfer) split out from execution, so
    device execution time can be measured separately from the axon-tunnel
    transfer cost.  Same lowering path run_bass_kernel_spmd takes under
    axon (bass2jax._bass_exec_p -> neuronx_cc_hook -> NEFF)."""

    def __init__(self, nc):
        import jax
        from jax.sharding import Mesh, PartitionSpec, NamedSharding
        try:
            from jax import shard_map
            def _shard_map(f, mesh, in_specs, out_specs):
                return shard_map(f, mesh=mesh, in_specs=in_specs,
                                 out_specs=out_specs, check_vma=False)
        except Exception:
            from jax.experimental.shard_map import shard_map
            def _shard_map(f, mesh, in_specs, out_specs):
                return shard_map(f, mesh=mesh, in_specs=in_specs,
                                 out_specs=out_specs, check_rep=False)
        from concourse import bass2jax, mybir as _mybir

        bass2jax.install_neuronx_cc_hook()
        self.jax = jax
        self.nc = nc
        pname = nc.partition_id_tensor.name if nc.partition_id_tensor else None
        in_names, out_names, out_avals, zero_outs = [], [], [], []
        for alloc in nc.m.functions[0].allocations:
            if not isinstance(alloc, _mybir.MemoryLocationSet):
                continue
            name = alloc.memorylocations[0].name
            if alloc.kind == "ExternalInput":
                if name != pname:
                    in_names.append(name)
            elif alloc.kind == "ExternalOutput":
                out_names.append(name)
                shape = tuple(alloc.tensor_shape)
                dtype = _mybir.dt.np(alloc.dtype)
                out_avals.append(jax.core.ShapedArray(shape, dtype))
                zero_outs.append(np.zeros(shape, dtype))
        self.in_names, self.out_names = in_names, out_names
        self.out_avals, self.zero_outs = out_avals, zero_outs
        n_params, n_outs = len(in_names), len(out_names)
        in_names_full = in_names + out_names
        if pname is not None:
            in_names_full.append(pname)

        def _body(*args):
            operands = list(args)
            if pname is not None:
                operands.append(bass2jax.partition_id_tensor())
            outs = bass2jax._bass_exec_p.bind(
                *operands, out_avals=tuple(out_avals),
                in_names=tuple(in_names_full), out_names=tuple(out_names),
                lowering_input_output_aliases=(), sim_require_finite=True,
                sim_require_nnan=True, nc=nc)
            return tuple(outs)

        devices = jax.devices()[:NCORES]
        assert len(devices) == NCORES
        mesh = Mesh(np.asarray(devices), ("core",))
        self.sharding = NamedSharding(mesh, PartitionSpec("core"))
        donate = tuple(range(n_params, n_params + n_outs))
        self.sharded = jax.jit(
            _shard_map(_body, mesh,
                       (PartitionSpec("core"),) * (n_params + n_outs),
                       (PartitionSpec("core"),) * n_outs),
            donate_argnums=donate, keep_unused=True)

    def put_inputs(self, in_maps):
        """Concat per-core inputs on axis 0 and transfer to the devices."""
        concat = [np.concatenate([np.asarray(m[nm]) for m in in_maps], axis=0)
                  for nm in self.in_names]
        dev = self.jax.device_put(concat, [self.sharding] * len(concat))
        self.jax.block_until_ready(dev)
        return dev

    def put_outbufs(self):
        """Fresh donated output buffers (kernel fully overwrites them)."""
        concat = [np.zeros((NCORES * z.shape[0], *z.shape[1:]), z.dtype)
                  for z in self.zero_outs]
        dev = self.jax.device_put(concat, [self.sharding] * len(concat))
        self.jax.block_until_ready(dev)
        return dev

    def execute(self, dev_in, dev_outs):
        """One kernel execution; returns device output arrays (async)."""
        return self.sharded(*dev_in, *dev_outs)

    def fetch(self, out_arrs):
        """Device->host; returns per-core result dicts."""
        host = [np.asarray(o) for o in out_arrs]
        return [{nm: host[i].reshape(NCORES, *self.out_avals[i].shape)[c]
                 for i, nm in enumerate(self.out_names)}
                for c in range(NCORES)]


def _get_runner():
    if "runner" not in _CACHE:
        _CACHE["runner"] = _PjrtRunner(_get_program())
    return _CACHE["runner"]


def _shared_inputs(W_ih0, W_hh0, b0, W_hr0, W_ih1, W_hh1, b1, W_hr1,
                   h_init, c_init, t_steps=T):
    perm = _gate_perm()
    Wi0, Wh0, b0p = W_ih0[perm], W_hh0[perm], b0[perm]
    Wi1, Wh1, b1p = W_ih1[perm], W_hh1[perm], b1[perm]

    sh = {
        "wa": _bf(Wi0[:, 0:128].T),
        "wxb": _bf(np.concatenate([Wi0[:, 128:135].T, b0p[None, :]], axis=0)),
        "wh0f": _bf((Wh0 @ W_hr0).T),
        "wh0i": _bf(Wh0.T),
        "wA1": _bf((Wi1 @ W_hr0).T),
        "wB1": _bf((Wh1 @ W_hr1).T),
        "wh1i": _bf(Wh1.T),
        "b1r": _bf(b1p[None, :]),
        "b1c": _f32(b1p.reshape(4, 128).T),
        "wr1": _bf(W_hr1.T),
        "h0i": _bf(np.broadcast_to(h_init[0].reshape(PJ, 1), (PJ, B))),
        "h1i": _bf(np.broadcast_to(h_init[1].reshape(PJ, 1), (PJ, B))),
        "one": _bf(np.ones((1, B), np.float32)),
        "c0i": _bf(np.broadcast_to(c_init[0].reshape(H, 1), (H, B))),
        "c1i": _bf(np.broadcast_to(c_init[1].reshape(H, 1), (H, B))),
        "pri": _f32(_length_priors_np()[:t_steps].T),
        "iot": _bf(np.broadcast_to(np.arange(64, dtype=np.float32)[None, :], (128, 64))),
        "idn": _f32(np.eye(64, dtype=np.float32)),
        "epb": _f32(np.full((128, 1), EPS, np.float32)),
    }
    return sh


def _core_inputs(inputs_seq, tokens, core, t_steps=T):
    xs = inputs_seq[:t_steps, core * B:(core + 1) * B, :]       # [T, 512, 135]
    xT = np.ascontiguousarray(np.transpose(xs, (0, 2, 1)))      # [T, 135, 512]
    toks = tokens[core * B:(core + 1) * B, :t_steps]            # [512, T]
    tokl = np.ascontiguousarray(
        np.transpose(toks.reshape(G, 128, t_steps), (1, 0, 2)).reshape(128, G * t_steps))
    return {
        "xa": _bf(xT[:, 0:128, :]),
        "xb": _bf(xT[:, 128:135, :]),
        "tok": _bf(tokl),
    }


def _make_in_maps(inputs_seq, W_ih0, W_hh0, b0, W_hr0, W_ih1, W_hh1, b1,
                  W_hr1, h_init, c_init, tokens):
    inputs_seq = np.asarray(inputs_seq, dtype=np.float32)
    tokens_np = np.asarray(tokens)
    sh = _shared_inputs(np.asarray(W_ih0, np.float32), np.asarray(W_hh0, np.float32),
                        np.asarray(b0, np.float32), np.asarray(W_hr0, np.float32),
                        np.asarray(W_ih1, np.float32), np.asarray(W_hh1, np.float32),
                        np.asarray(b1, np.float32), np.asarray(W_hr1, np.float32),
                        np.asarray(h_init, np.float32), np.asarray(c_init, np.float32))
    in_maps = []
    for core in range(NCORES):
        m = dict(sh)
        m.update(_core_inputs(inputs_seq, tokens_np, core))
        in_maps.append(m)
    return in_maps


def _unshard(results):
    ents = np.empty((NB, T), np.float32)
    lps = np.empty((NB, T), np.float32)
    for core in range(NCORES):
        r = results[core]
        e = r["ents"].reshape(128, G, T).transpose(1, 0, 2).reshape(B, T)
        l = r["lps"].reshape(128, G, T).transpose(1, 0, 2).reshape(B, T)
        ents[core * B:(core + 1) * B] = e
        lps[core * B:(core + 1) * B] = l
    return ents, lps


def timed_execs(n_reps):
    """Run the already-staged kernel n_reps times back-to-back on the
    device-resident inputs and return the total wall-clock ns.  Executions
    serialize on the NeuronCores (each rep's donated output buffers are the
    previous rep's outputs), so wall/n_reps upper-bounds the per-execution
    device time; differencing two rep counts cancels the constant dispatch
    round-trip latency of the axon tunnel."""
    import time as _time
    runner = _CACHE["runner"]
    dev_in = _CACHE["dev_in"]
    outs = runner.execute(dev_in, runner.put_outbufs())  # warm/stage
    runner.jax.block_until_ready(outs)
    t0 = _time.perf_counter()
    for _ in range(n_reps):
        outs = runner.execute(dev_in, outs)
    runner.jax.block_until_ready(outs)
    return (_time.perf_counter() - t0) * 1e9


def kernel(inputs_seq, W_ih0, W_hh0, b0, W_hr0, W_ih1, W_hh1, b1, W_hr1,
           h_init, c_init, tokens, _trace=False):
    in_maps = _make_in_maps(inputs_seq, W_ih0, W_hh0, b0, W_hr0, W_ih1,
                            W_hh1, b1, W_hr1, h_init, c_init, tokens)
    import time as _time
    try:
        runner = _get_runner()
        dev_in = runner.put_inputs(in_maps)
        _CACHE["dev_in"] = dev_in
        _t0 = _time.perf_counter()
        out_arrs = runner.execute(dev_in, runner.put_outbufs())
        runner.jax.block_until_ready(out_arrs)
        _CACHE["exec_wall_ns"] = (_time.perf_counter() - _t0) * 1e9
        results = runner.fetch(out_arrs)
    except Exception:
        # fall back to the stock SPMD path
        nc = _get_program()
        _t0 = _time.perf_counter()
        res = run_bass_kernel_spmd(nc, in_maps, core_ids=list(range(NCORES)))
        _CACHE["exec_wall_ns"] = (_time.perf_counter() - _t0) * 1e9
        results = res.results
    return _unshard(results)



# revision 50
# speedup vs baseline: 1.1381x; 1.1381x over previous
"""Trainium2 Bass kernel for DSOAgent sampling (2-layer projected LSTM decode).

Math per step t (batch n, per core n=512):
  L0: gates = W_ih0 @ x_t + W_hh0 @ h0 + b0 ; c0' = sig(f)*c0 + sig(i)*tanh(g)
      h0' = (sig(o)*tanh(c0')) @ W_hr0.T
  L1: same with h0' as input -> h1'
  logits = h1' + prior[t];  p = softmax(logits)+eps (renorm ~1)
  ent[:,t] = -sum p*log(p);  lp[:,t] = log(p)[tokens[:,t]]

Sharding: pure data parallel, batch 4096 -> 8 cores x 512.

Device layout: feature-major [feat_part, batch_free] for the recurrence;
softmax done batch-major after a PE transpose.  Gate banks are ordered
(f, i, o, g) so one batched Sigmoid covers f,i,o.

Performance structure (what made it fast):
- Projection fusion on host: W_hh0@W_hr0, W_ih1@W_hr0, W_hh1@W_hr1 are
  pre-multiplied, so the recurrent state is the 128-dim pre-projection
  activation (hp) and no psum->SBUF state copies sit on the loop.  Step 0
  uses the unfused 64-dim form to consume h_init/c_init exactly.
- Biases never cost matmuls: layer 0's rides the ones-row in the xb
  moving chunk; layer 1's is applied as the per-partition ACT bias of
  per-gate full-width sigmoid/tanh ops (dropping the 4 K=1 bias matmuls
  per step cut ~1000 instructions and ~0.4ms of real-HW marginal time,
  and lets the cell update start as soon as the f-gate sigmoid lands).
- Each layer's gates use a 3-bank f/i/o psum pool + 1-bank g pool; the
  hr1/transpose/backlog scratch is time-multiplexed into the consumed
  g bank, so all 8 psum banks are productive and the sigma-critical banks
  recycle immediately after the Sigmoid reads them.
- The whole recurrence (matmuls, sigma via 3D strided APs, tanh, cell
  update, h_pre) is split into two independent 256-column batch halves
  that pipeline through PE->ACT->DVE; matmul groups are uniform
  half-width so psum accumulation-group tracking stays valid.
- Recurrence-chain ops are priority-boosted (tc.high_priority; L0 above
  L1) so the scheduler never wedges off-chain work into the serial loop.
- The softmax tail is computed per 32-step block WITHOUT materializing p
  or log(p): ent = lnZ - sum(x*e^x)/Z and lp_tok = ln(p_tok + eps) with
  p_tok = (sum_v onehot(tok)*e^x)/Z gathered by an is_equal mask and a
  3D tensor_reduce.  This replaces the per-(step,group) tiny
  scalar_tensor_tensor ops with a few wide DVE ops (drip-issued in
  8-step chunks across the next block) and shrinks the per-block ACT
  phase to three
  back-to-back ops (Exp, Ln(Z), Ln(p_tok)) under one exp/ln table phase
  (priority-boosted so no sigmoid wedges in and thrashes the table).
  bf16 e^x underflow at x<-87 reproduces the 1e-10 log clamp exactly.
- Full-width (512-col) matmuls and 4-step-batched xa/xb DMAs: the sim
  says half-width pipelines marginally better, but real-HW marginal
  per-exec time drops ~0.25ms from the halved PE instruction count
  (per-exec instruction fetch dominates the sim-vs-HW gap).
- bf16 matmul operands and cell state (forget-gate decay bounds drift;
  validated vs fp32), fp32 psum/logits so entropy/log-prob errors stay
  ~1e-3 relative (tolerance 2e-2).

Measurement: NTFF profiling is unavailable under axon here, so the "HW
exec time" is the marginal wall-clock per execution: back-to-back reps
on device-resident inputs (donated-output chaining serializes reps on
the cores), differenced between two rep counts to cancel the constant
dispatch round-trip.  Upper-bounds true device time (includes per-exec
NEFF launch overhead, ~0.8ms of the ~2ms).
"""

import os
from contextlib import ExitStack

import ml_dtypes
import numpy as np

import concourse.bass as bass
import concourse.tile as tile
from concourse import bacc, mybir
from concourse.bass_utils import run_bass_kernel_spmd
from concourse.tile_rust import add_dep_helper as _add_dep_raw


def add_dep_helper(frm, to, sync=True, reason=""):
    _add_dep_raw(getattr(frm, "ins", frm), getattr(to, "ins", to),
                 sync=sync, reason=reason)

F32 = mybir.dt.float32
BF16 = mybir.dt.bfloat16
AF = mybir.ActivationFunctionType
OP = mybir.AluOpType

T = 128          # decode steps
NB = 4096        # total batch
IN = 135         # input feature size
H = 128          # LSTM hidden
PJ = 64          # proj size / vocab
NCORES = 8
B = NB // NCORES  # per-core batch = 512
G = B // 128      # batch groups of 128 partitions = 4
KBLK = int(os.environ.get("K_KBLK", "32"))  # steps per softmax block
NBLK = T // KBLK
EPS = 1e-10

# batch-split factors per engine (env-settable for TimelineSim A/B runs);
# the recurrence is issued as independent per-slice chains that pipeline.
SPLIT_MM = int(os.environ.get("K_SPLIT_MM", "1"))
SPLIT_ACT = int(os.environ.get("K_SPLIT_ACT", "2"))
SPLIT_DVE = int(os.environ.get("K_SPLIT_DVE", "2"))
SPLIT_ACT1 = int(os.environ.get("K_SPLIT_ACT1", str(SPLIT_ACT)))
SPLIT_DVE1 = int(os.environ.get("K_SPLIT_DVE1", str(SPLIT_DVE)))
B1ACT = os.environ.get("K_B1ACT", "1") == "1"    # L1 bias via ACT bias operand
L0GATE = os.environ.get("K_L0GATE", "0") == "1"  # per-gate full-width L0 sigmoid
T2 = os.environ.get("K_T2", "0") == "1"          # 2-step packed logits transpose
HPF = os.environ.get("K_HPF", "0") == "1"        # full-width end-of-chain hp mults


def _slices(n):
    w = 512 // n
    return [slice(i * w, (i + 1) * w) for i in range(n)]

# PyTorch gate order i,f,g,o; we reorder rows to (f, i, o, g) so the three
# sigmoid gates occupy adjacent psum banks.
def _gate_perm():
    i = np.arange(0, H)
    f = np.arange(H, 2 * H)
    g = np.arange(2 * H, 3 * H)
    o = np.arange(3 * H, 4 * H)
    return np.concatenate([f, i, o, g])


def _bf(x):
    return np.ascontiguousarray(x.astype(ml_dtypes.bfloat16))


def _f32(x):
    return np.ascontiguousarray(x.astype(np.float32))


def _length_priors_np():
    t = np.arange(T, dtype=np.float32)
    idx = np.arange(PJ)
    zero_mask = ((idx >= 0) & (idx < 32)).astype(np.float32)
    two_mask = ((idx >= 48) & (idx < 64)).astype(np.float32)
    pen_short = np.where(t < 64.0, -((64.0 - t) ** 2) / 16.0, 0.0).astype(np.float32)
    pen_long = np.where(t > 64.0, -((t - 64.0) ** 2) / 16.0, 0.0).astype(np.float32)
    return pen_short[:, None] * zero_mask[None, :] + pen_long[:, None] * two_mask[None, :]


def build_program(t_steps=T, kblk=KBLK):
    """Build and compile the single-core Bass program (same program runs on
    all 8 cores, SPMD over the batch)."""
    nblk = t_steps // kblk
    nc = bacc.Bacc(
        "TRN2",
        target_bir_lowering=False,
        debug=False,
        enable_asserts=False,
        num_devices=1,
    )

    # ---- DRAM I/O ----
    d_xa = nc.dram_tensor("xa", [t_steps, 128, B], BF16, kind="ExternalInput").ap()
    d_xb = nc.dram_tensor("xb", [t_steps, 7, B], BF16, kind="ExternalInput").ap()
    d_wa = nc.dram_tensor("wa", [128, 512], BF16, kind="ExternalInput").ap()
    d_wxb = nc.dram_tensor("wxb", [8, 512], BF16, kind="ExternalInput").ap()
    d_wh0f = nc.dram_tensor("wh0f", [128, 512], BF16, kind="ExternalInput").ap()
    d_wh0i = nc.dram_tensor("wh0i", [64, 512], BF16, kind="ExternalInput").ap()
    d_wA1 = nc.dram_tensor("wA1", [128, 512], BF16, kind="ExternalInput").ap()
    d_wB1 = nc.dram_tensor("wB1", [128, 512], BF16, kind="ExternalInput").ap()
    d_wh1i = nc.dram_tensor("wh1i", [64, 512], BF16, kind="ExternalInput").ap()
    d_b1r = nc.dram_tensor("b1r", [1, 512], BF16, kind="ExternalInput").ap()
    d_b1c = nc.dram_tensor("b1c", [128, 4], F32, kind="ExternalInput").ap()
    d_wr1 = nc.dram_tensor("wr1", [128, 64], BF16, kind="ExternalInput").ap()
    d_h0i = nc.dram_tensor("h0i", [64, B], BF16, kind="ExternalInput").ap()
    d_h1i = nc.dram_tensor("h1i", [64, B], BF16, kind="ExternalInput").ap()
    d_c0i = nc.dram_tensor("c0i", [128, B], BF16, kind="ExternalInput").ap()
    d_c1i = nc.dram_tensor("c1i", [128, B], BF16, kind="ExternalInput").ap()
    d_tok = nc.dram_tensor("tok", [128, G * t_steps], BF16, kind="ExternalInput").ap()
    d_pri = nc.dram_tensor("pri", [128, t_steps], F32, kind="ExternalInput").ap()
    d_iot = nc.dram_tensor("iot", [128, 64], BF16, kind="ExternalInput").ap()
    d_idn = nc.dram_tensor("idn", [128, 128], F32, kind="ExternalInput").ap()
    d_epb = nc.dram_tensor("epb", [128, 1], F32, kind="ExternalInput").ap()
    d_one = nc.dram_tensor("one", [1, B], BF16, kind="ExternalInput").ap()
    d_ent = nc.dram_tensor("ents", [128, G * t_steps], F32, kind="ExternalOutput").ap()
    d_lp = nc.dram_tensor("lps", [128, G * t_steps], F32, kind="ExternalOutput").ap()

    with tile.TileContext(nc) as tc, ExitStack() as ctx:
        _build_tile(ctx, tc, t_steps, kblk, nblk, dict(
            xa=d_xa, xb=d_xb, wa=d_wa, wxb=d_wxb, wh0f=d_wh0f, wh0i=d_wh0i,
            wA1=d_wA1, wB1=d_wB1, wh1i=d_wh1i, b1r=d_b1r, b1c=d_b1c, wr1=d_wr1, h0i=d_h0i, h1i=d_h1i, c0i=d_c0i, c1i=d_c1i, one=d_one,
            tok=d_tok, pri=d_pri, iot=d_iot, idn=d_idn, epb=d_epb, ent=d_ent, lp=d_lp,
        ))

    nc.compile()
    return nc


def _build_tile(ctx, tc, t_steps, kblk, nblk, io):
    nc = tc.nc

    cst = ctx.enter_context(tc.tile_pool(name="cst", bufs=1))
    st = ctx.enter_context(tc.tile_pool(name="st", bufs=1))
    wk = ctx.enter_context(tc.tile_pool(name="wk", bufs=3))
    wkx = ctx.enter_context(tc.tile_pool(name="wkx", bufs=3))
    pgL0f = ctx.enter_context(tc.tile_pool(name="pgL0f", bufs=1, space="PSUM"))
    pgL0g = ctx.enter_context(tc.tile_pool(name="pgL0g", bufs=1, space="PSUM"))
    pgL1f = ctx.enter_context(tc.tile_pool(name="pgL1f", bufs=1, space="PSUM"))
    pgL1g = ctx.enter_context(tc.tile_pool(name="pgL1g", bufs=1, space="PSUM"))

    def load_const(name, shape, dt):
        t_ = cst.tile(shape, dt, tag=name)
        nc.sync.dma_start(t_[:], io[name][:])
        return t_

    wa = load_const("wa", [128, 512], BF16)
    wxb = load_const("wxb", [8, 512], BF16)
    wh0f = load_const("wh0f", [128, 512], BF16)
    wh0i = load_const("wh0i", [64, 512], BF16)
    wA1 = load_const("wA1", [128, 512], BF16)
    wB1 = load_const("wB1", [128, 512], BF16)
    wh1i = load_const("wh1i", [64, 512], BF16)
    b1r = load_const("b1r", [1, 512], BF16)
    b1c = load_const("b1c", [128, 4], F32)
    one_t = load_const("one", [1, B], BF16)
    wr1 = load_const("wr1", [128, 64], BF16)
    tok = load_const("tok", [128, G * t_steps], BF16)
    pri = load_const("pri", [128, t_steps], F32)
    iot = load_const("iot", [128, 64], BF16)
    idn = load_const("idn", [128, 128], F32)
    epb = load_const("epb", [128, 1], F32)

    # persistent state (double-buffered across steps)
    # sxb: [xb(7); ones(1)] input chunk; hp0: layer-0 pre-projection state;
    # h1: [h1(64); ones(1)] layer-1 projected state; h0i/h1i initial h states
    # xa/xb stream in XB4-step batches (one DMA per tensor per XB4 steps)
    XB4 = int(os.environ.get("K_XB4", "4"))
    sxb = [st.tile([8, XB4 * B], BF16, tag=f"sxb_{k}", name=f"sxb_{k}")
           for k in range(2)]
    hp0s = [st.tile([128, B], BF16, tag=f"hp0s_{k}", name=f"hp0s_{k}") for k in range(2)]
    hp1s = [st.tile([128, B], BF16, tag=f"hp1s_{k}", name=f"hp1s_{k}") for k in range(2)]
    h0i = st.tile([64, B], BF16, tag="h0i", name="h0i")
    h1i = st.tile([64, B], BF16, tag="h1i", name="h1i")
    c0 = st.tile([128, B], BF16, tag="c0", name="c0")
    c1 = st.tile([128, B], BF16, tag="c1", name="c1")
    for k in range(2):
        for j in range(XB4):
            nc.sync.dma_start(sxb[k][7:8, j * B:(j + 1) * B], io["one"][:])
    nc.sync.dma_start(h0i[:], io["h0i"][:])
    nc.sync.dma_start(h1i[:], io["h1i"][:])
    nc.sync.dma_start(c0[:], io["c0i"][:])
    nc.sync.dma_start(c1[:], io["c1i"][:])

    # softmax block buffers.  Per block (16 steps x 4 groups x 64 vocab):
    #   e = exp(x); Z = sum_v e; ent = lnZ - sum_v(x*e)/Z
    #   lp_tok = ln(exp(x_tok - lnZ) + eps)  (x_tok via one-hot gather-reduce)
    backlog = [st.tile([128, kblk * 256], F32, tag=f"bl_{k}", name=f"bl_{k}") for k in range(2)]
    e_blk = st.tile([128, kblk * 256], BF16, tag="e_blk", name="e_blk")
    q_blk = st.tile([128, kblk * 256], BF16, tag="q_blk", name="q_blk")
    eq_blk = st.tile([128, kblk * 256], BF16, tag="eq_blk", name="eq_blk")
    ql_blk = q_blk  # shared scratch: s2 consumes q before ql writes (drip order)
    zs = st.tile([128, kblk * G], F32, tag="zs", name="zs")
    rz = st.tile([128, kblk * G], F32, tag="rz", name="rz")
    s2n = st.tile([128, kblk * G], F32, tag="s2n", name="s2n")
    ptk = st.tile([128, kblk * G], F32, tag="ptk", name="ptk")
    lnz = st.tile([128, kblk * G], F32, tag="lnz", name="lnz")
    pt2 = st.tile([128, kblk * G], F32, tag="pt2", name="pt2")
    se = st.tile([128, kblk * G], F32, tag="se", name="se")
    ent_o = st.tile([128, G * t_steps], F32, tag="ent_o", name="ent_o")
    lp_o = st.tile([128, G * t_steps], F32, tag="lp_o", name="lp_o")

    last_act = [None]   # last recurrence ACT op of current block
    deferred = []       # phase tail ops, drip-issued into the next block

    def act(*a, **k):
        op = nc.scalar.activation(*a, **k)
        last_act[0] = op
        return op

    def _sg4(tile_, off=0):
        """[128, kblk*256] -> [128, s, g, v] 4D view."""
        return tile_[:].rearrange("p (s g v) -> p s g v", g=G, v=64)

    def _sg(tile_):
        """[128, kblk*G] -> [128, s, g] view (s-major, matching reduces)."""
        return tile_[:].rearrange("p (s g) -> p s g", g=G)

    def _cols(out_tile, blk):
        """[128, s, g] strided view into out_tile's (g*T + blk*kblk + s) cols."""
        v = out_tile[:].rearrange("p (g t) -> p g t", t=t_steps)
        v = v[:, :, blk * kblk:(blk + 1) * kblk]
        return v.rearrange("p g s -> p s g")

    def _tok4(blk, s0, ns):
        """tokens for steps [blk*kblk+s0, +ns) as [128, ns, g, v(bcast)]."""
        v = tok[:].rearrange("p (g t) -> p g t", t=t_steps)
        v = v[:, :, blk * kblk + s0:blk * kblk + s0 + ns]
        v = v.rearrange("p g s -> p s g")
        return v.rearrange("p s (g o) -> p s g o", o=1).broadcast_to(
            [128, ns, G, 64])

    def softmax_phase(blk):
        """Emit the exp-table ACT group: Exp for block blk plus the two Ln
        ops finishing block blk-1 (all inputs pre-computed by the drip, so
        the three ops run back-to-back under one table phase)."""
        pbk = blk % 2
        with tc.high_priority(70000):
            exp_op = nc.scalar.activation(e_blk[:], backlog[pbk][:], AF.Exp)
            if os.environ.get("K_EXP_NODEP", "0") != "1":
                add_dep_helper(exp_op, last_act[0], sync=False,
                               reason="exp after recurrence ACT of block")
            if blk > 0:
                # lnZ of block blk-1 (zs ready from last block's drip)
                op = nc.scalar.activation(lnz[:], zs[:], AF.Ln)
                add_dep_helper(op, exp_op, sync=False, reason="lnz after exp")
                # lp(blk-1) = ln(p_tok + eps); p_tok = (sum eq*e)/Z from drip
                op2 = nc.scalar.activation(_cols(lp_o, blk - 1), _sg(pt2),
                                           AF.Ln, bias=epb[:])
                add_dep_helper(op2, op, sync=False, reason="lp after lnz")
        _emit_drip(blk)

    def _emit_drip(blk):
        """Deferred DVE work: ent combine for blk-1, then reduces for blk,
        split into s-halves so no single drip op overruns a step's DVE
        slack."""
        pbk = blk % 2
        nch = max(1, kblk // 8)                # 8-step drip chunks
        hw_, hc = kblk // nch, kblk // nch * 256
        hg = kblk // nch * G

        def h4(tile_, hs):
            return tile_[:, hs * hc:(hs + 1) * hc].rearrange(
                "p (s g v) -> p s g v", g=G, v=64)

        def hsg(tile_, hs):
            return tile_[:, hs * hg:(hs + 1) * hg].rearrange(
                "p (s g) -> p s g", g=G)

        if blk > 0:
            def _ent(blk=blk):
                # ent(blk-1) = lnz + (-s2)/Z  (lnz from this phase's ACT)
                nc.vector.tensor_tensor(se[:], s2n[:], rz[:], OP.mult)
                nc.vector.tensor_tensor(_cols(ent_o, blk - 1), _sg(lnz),
                                        _sg(se), OP.add)
            deferred.append(_ent)

        for hs in range(nch):
            def _z(hs=hs):
                nc.vector.tensor_reduce(hsg(zs, hs), h4(e_blk, hs),
                                        axis=mybir.AxisListType.X, op=OP.add)
                if hs == nch - 1:
                    nc.vector.reciprocal(rz[:], zs[:])
            deferred.append(_z)
        for hs in range(nch):
            def _q(hs=hs, pbk=pbk):
                nc.vector.tensor_tensor(
                    q_blk[:, hs * hc:(hs + 1) * hc],
                    backlog[pbk][:, hs * hc:(hs + 1) * hc],
                    e_blk[:, hs * hc:(hs + 1) * hc], OP.mult)
            deferred.append(_q)
        for hs in range(nch):
            def _s2(hs=hs):
                nc.vector.tensor_reduce(hsg(s2n, hs), h4(q_blk, hs),
                                        axis=mybir.AxisListType.X, op=OP.add,
                                        negate=True)
            deferred.append(_s2)
        for hs in range(nch):
            def _eq(hs=hs, blk=blk):
                iot4 = iot[:].rearrange("p (a b v) -> p a b v",
                                        a=1, b=1).broadcast_to(
                    [128, hw_, G, 64])
                tok4 = _tok4(blk, hs * hw_, hw_)
                nc.vector.tensor_tensor(h4(eq_blk, hs), iot4, tok4,
                                        OP.is_equal)
            deferred.append(_eq)
        for hs in range(nch):
            def _ql(hs=hs):
                nc.vector.tensor_tensor(
                    ql_blk[:, hs * hc:(hs + 1) * hc],
                    eq_blk[:, hs * hc:(hs + 1) * hc],
                    e_blk[:, hs * hc:(hs + 1) * hc], OP.mult)
            deferred.append(_ql)
        for hs in range(nch):
            def _ptk(hs=hs):
                nc.vector.tensor_reduce(hsg(ptk, hs), h4(ql_blk, hs),
                                        axis=mybir.AxisListType.X, op=OP.add)
            deferred.append(_ptk)

        def _pt2():
            nc.vector.tensor_tensor(pt2[:], ptk[:], rz[:], OP.mult)
        deferred.append(_pt2)

    def final_phase(blk):
        """Finish block blk's outputs at the end of the program."""
        op = nc.scalar.activation(lnz[:], zs[:], AF.Ln)
        add_dep_helper(op, last_act[0], sync=False, reason="final lnz")
        op2 = nc.scalar.activation(_cols(lp_o, blk), _sg(pt2), AF.Ln,
                                   bias=epb[:])
        add_dep_helper(op2, op, sync=False, reason="final lp")
        nc.vector.tensor_tensor(se[:], s2n[:], rz[:], OP.mult)
        nc.vector.tensor_tensor(_cols(ent_o, blk), _sg(lnz), _sg(se), OP.add)

    for t in range(t_steps):
        p_, pn = t % 2, (t + 1) % 2
        blk, s_in = t // kblk, t % kblk

        # input DMAs (batched: one DMA covers 4 steps)
        g4, s4 = t // XB4, t % XB4
        if s4 == 0:
            xa4_cur = wkx.tile([128, XB4 * B], BF16, tag="xa", name="xa")
            nc.sync.dma_start(
                xa4_cur[:].rearrange("p (t b) -> p t b", t=XB4),
                io["xa"][t:t + XB4].rearrange("t p b -> p t b"))
            nc.sync.dma_start(
                sxb[g4 % 2][0:7, :].rearrange("p (t b) -> p t b", t=XB4),
                io["xb"][t:t + XB4].rearrange("t p b -> p t b"))
        xa_tile, sxb_tile, xoff = xa4_cur, sxb[g4 % 2], s4 * B

        mm_slices = _slices(SPLIT_MM)
        act_slices = _slices(SPLIT_ACT)
        dve_slices = _slices(SPLIT_DVE)

        def gsl(g, sl):
            return slice(g * 512 + sl.start, g * 512 + sl.stop)

        # ---- layer 0 gates: psum banks (f, i, o) + (g) ----
        gf = pgL0f.tile([128, 1536], F32, tag="gL0f", name="gL0f")
        gg = pgL0g.tile([128, 512], F32, tag="gL0g", name="gL0g")
        with tc.high_priority(60000):
            for hsl in mm_slices:
                for m in range(4):
                    out = gf[:, m * 512:(m + 1) * 512] if m < 3 else gg[:]
                    msl = slice(m * 128, (m + 1) * 128)
                    nc.tensor.matmul(
                        out[:, hsl], wa[:, msl],
                        xa_tile[:, xoff + hsl.start:xoff + hsl.stop],
                        start=True, stop=False)
                    nc.tensor.matmul(
                        out[:, hsl], wxb[:, msl],
                        sxb_tile[:, xoff + hsl.start:xoff + hsl.stop],
                        start=False, stop=False)
                    if t == 0:
                        nc.tensor.matmul(out[:, hsl], wh0i[:, msl], h0i[:, hsl],
                                         start=False, stop=True)
                    else:
                        nc.tensor.matmul(out[:, hsl], wh0f[:, msl],
                                         hp0s[p_][:, hsl], start=False, stop=True)

        sfio = wk.tile([128, 1536], BF16, tag="sfio", name="sfio")
        with tc.high_priority(60000):
            gf3 = gf[:].rearrange("p (b n) -> p b n", n=512)
            sf3 = sfio[:].rearrange("p (b n) -> p b n", n=512)
            first_sig = None
            if L0GATE:
                for gi in range(3):
                    op = nc.scalar.activation(
                        sfio[:, gi * 512:(gi + 1) * 512],
                        gf[:, gi * 512:(gi + 1) * 512], AF.Sigmoid)
                    if first_sig is None:
                        first_sig = op
            else:
                for sl in act_slices:
                    op = nc.scalar.activation(sf3[:, :, sl], gf3[:, :, sl],
                                              AF.Sigmoid)
                    if first_sig is None:
                        first_sig = op
        # no hard exp->sigma gate: letting the scheduler interleave the next
        # block's recurrence through the ln/exp phase keeps PE warm; costs a
        # couple of extra ACT table loads per block (counted: 36 vs 24 total)
        # but nets faster overall.
        last_act[0] = first_sig
        tg = wk.tile([128, 512], BF16, tag="tg", name="tg")
        m0 = wk.tile([128, 512], BF16, tag="m0", name="m0")
        t1 = wk.tile([128, 512], BF16, tag="t1", name="t1")
        tc0 = wk.tile([128, 512], BF16, tag="tc0", name="tc0")
        hp0 = hp0s[pn]
        with tc.high_priority(60000):
            for sl in act_slices:
                act(tg[:, sl], gg[:, sl], AF.Tanh)
            for sl in dve_slices:
                nc.vector.tensor_tensor(
                    m0[:, sl], sfio[:, gsl(1, sl)], tg[:, sl], OP.mult)
                nc.vector.tensor_tensor(
                    t1[:, sl], sfio[:, gsl(0, sl)], c0[:, sl], OP.mult)
                nc.vector.tensor_tensor(c0[:, sl], m0[:, sl], t1[:, sl], OP.add)
            for sl in act_slices:
                act(tc0[:, sl], c0[:, sl], AF.Tanh)
            for sl in (_slices(1) if HPF else dve_slices):
                nc.vector.tensor_tensor(
                    hp0[:, sl], sfio[:, gsl(2, sl)], tc0[:, sl], OP.mult)

        # ---- layer 1 (input side fused with W_hr0) ----
        gf2 = pgL1f.tile([128, 1536], F32, tag="gL1f", name="gL1f")
        gg2 = pgL1g.tile([128, 512], F32, tag="gL1g", name="gL1g")
        with tc.high_priority(50000):
            for hsl in mm_slices:
                for m in ([3, 0, 1, 2] if os.environ.get("K_GFIRST", "0") == "1"
                          else range(4)):
                    out = gf2[:, m * 512:(m + 1) * 512] if m < 3 else gg2[:]
                    msl = slice(m * 128, (m + 1) * 128)
                    if not B1ACT:
                        nc.tensor.matmul(out[:, hsl], b1r[:, msl], one_t[:, hsl],
                                         start=True, stop=False)
                    if t == 0:
                        nc.tensor.matmul(out[:, hsl], wh1i[:, msl], h1i[:, hsl],
                                         start=B1ACT, stop=False)
                    else:
                        nc.tensor.matmul(out[:, hsl], wB1[:, msl],
                                         hp1s[p_][:, hsl], start=B1ACT, stop=False)
                    nc.tensor.matmul(out[:, hsl], wA1[:, msl], hp0[:, hsl],
                                     start=False, stop=True)

        sfio1 = wk.tile([128, 1536], BF16, tag="sfio1", name="sfio1")
        with tc.high_priority(50000):
            gf23 = gf2[:].rearrange("p (b n) -> p b n", n=512)
            sf13 = sfio1[:].rearrange("p (b n) -> p b n", n=512)
            tg1 = wk.tile([128, 512], BF16, tag="tg1", name="tg1")
            m1 = wk.tile([128, 512], BF16, tag="m1", name="m1")
            t11 = wk.tile([128, 512], BF16, tag="t11", name="t11")
            tc1 = wk.tile([128, 512], BF16, tag="tc1", name="tc1")
            hp1 = hp1s[pn]
            if B1ACT:
                # per-gate full-width sigmoids so each takes its own b1 bias;
                # tanh(g) takes the g-gate bias the same way
                for gi in range(3):
                    act(sfio1[:, gi * 512:(gi + 1) * 512],
                        gf2[:, gi * 512:(gi + 1) * 512], AF.Sigmoid,
                        bias=b1c[:, gi:gi + 1])
                act(tg1[:], gg2[:], AF.Tanh, bias=b1c[:, 3:4])
            else:
                for sl in _slices(SPLIT_ACT1):
                    if os.environ.get("K_GFIRST", "0") == "1":
                        act(tg1[:, sl], gg2[:, sl], AF.Tanh)
                        act(sf13[:, :, sl], gf23[:, :, sl], AF.Sigmoid)
                    else:
                        act(sf13[:, :, sl], gf23[:, :, sl], AF.Sigmoid)
                        act(tg1[:, sl], gg2[:, sl], AF.Tanh)
            for sl in _slices(SPLIT_DVE1):
                nc.vector.tensor_tensor(
                    m1[:, sl], sfio1[:, gsl(1, sl)], tg1[:, sl], OP.mult)
                nc.vector.tensor_tensor(
                    t11[:, sl], sfio1[:, gsl(0, sl)], c1[:, sl], OP.mult)
                nc.vector.tensor_tensor(c1[:, sl], m1[:, sl], t11[:, sl], OP.add)
            for sl in _slices(SPLIT_ACT1):
                act(tc1[:, sl], c1[:, sl], AF.Tanh)
            for sl in (_slices(1) if HPF else _slices(SPLIT_DVE1)):
                nc.vector.tensor_tensor(
                    hp1[:, sl], sfio1[:, gsl(2, sl)], tc1[:, sl], OP.mult)

        if T2:
            # 2-step packed logits path: even steps park h1'+prior in rows
            # 0:64 of a shared [128,512] SBUF tile, odd steps in rows 64:128
            # (hr1 writes psum partitions 64:128 directly so the prior-add
            # stays lane-aligned); the pair is transposed and copied to the
            # backlog once, halving transpose/copy instructions.
            po = (t % 2) * 64
            aps_ = gg2[po:po + 64, :]
            nc.tensor.matmul(aps_, wr1[:], hp1[:], start=True, stop=True,
                             skip_group_check=True)
            if t % 2 == 0:
                lgt2 = wk.tile([128, 512], F32, tag="lgt", name="lgt")
            nc.vector.tensor_scalar(lgt2[po:po + 64, :], aps_,
                                    pri[po:po + 64, t:t + 1], None, OP.add)
            if t % 2 == 1:
                bps = gg2[0:128, :]
                for g in range(G):
                    nc.tensor.matmul(
                        bps[:, g * 128:(g + 1) * 128],
                        lgt2[:, g * 128:(g + 1) * 128], idn[:],
                        is_transpose=True, skip_group_check=True)
                src_ = bps.rearrange("p (g s v) -> p s g v", s=2, v=64)
                dst_ = backlog[blk % 2][:, (s_in - 1) * 256:(s_in + 1) * 256]
                dst_ = dst_.rearrange("p (s g v) -> p s g v", g=G, v=64)
                nc.vector.tensor_copy(dst_, src_)
        else:
            aps_ = gg2[0:64, :]
            nc.tensor.matmul(aps_, wr1[:], hp1[:], start=True, stop=True,
                             skip_group_check=True)
            lgt = wk.tile([64, 512], F32, tag="lgt", name="lgt")
            nc.vector.tensor_scalar(lgt[:], aps_, pri[0:64, t:t + 1], None,
                                    OP.add)
            bps = gg2[0:128, 0:256]
            for g in range(G):
                nc.tensor.matmul(
                    bps[:, g * 64:(g + 1) * 64], lgt[:, g * 128:(g + 1) * 128],
                    idn[0:64, 0:64], is_transpose=True, skip_group_check=True)
            nc.vector.tensor_copy(
                backlog[blk % 2][:, s_in * 256:(s_in + 1) * 256], bps)

        # drip-issue deferred phase-tail ops (after the step body so they
        # rank below this step's tail ops in the scheduler's tie-breaks)
        if deferred:
            deferred.pop(0)()

        if s_in == kblk - 1:
            softmax_phase(blk)

    # final block: drain deferred, then finish its ent/lp outputs
    while deferred:
        deferred.pop(0)()
    final_phase(nblk - 1)

    nc.sync.dma_start(io["ent"][:], ent_o[:])
    nc.sync.dma_start(io["lp"][:], lp_o[:])


# ---------------------------------------------------------------------------
# host side
# ---------------------------------------------------------------------------

_CACHE = {}


def _get_program():
    if "nc" not in _CACHE:
        _CACHE["nc"] = build_program()
    return _CACHE["nc"]


class _PjrtRunner:
    """Executes the compiled Bass module on the 8 NeuronCores via PJRT with
    the input staging (host->device transfer) split out from execution, so
    device execution time can be measured separately from the axon-tunnel
    transfer cost.  Same lowering path run_bass_kernel_spmd takes under
    axon (bass2jax._bass_exec_p -> neuronx_cc_hook -> NEFF)."""

    def __init__(self, nc):
        import jax
        from jax.sharding import Mesh, PartitionSpec, NamedSharding
        try:
            from jax import shard_map
            def _shard_map(f, mesh, in_specs, out_specs):
                return shard_map(f, mesh=mesh, in_specs=in_specs,
                                 out_specs=out_specs, check_vma=False)
        except Exception:
            from jax.experimental.shard_map import shard_map
            def _shard_map(f, mesh, in_specs, out_specs):
                return shard_map(f, mesh=mesh, in_specs=in_specs,
                                 out_specs=out_specs, check_rep=False)
        from concourse import bass2jax, mybir as _mybir

        bass2jax.install_neuronx_cc_hook()
        self.jax = jax
        self.nc = nc
        pname = nc.partition_id_tensor.name if nc.partition_id_tensor else None
        in_names, out_names, out_avals, zero_outs = [], [], [], []
        for alloc in nc.m.functions[0].allocations:
            if not isinstance(alloc, _mybir.MemoryLocationSet):
                continue
            name = alloc.memorylocations[0].name
            if alloc.kind == "ExternalInput":
                if name != pname:
                    in_names.append(name)
            elif alloc.kind == "ExternalOutput":
                out_names.append(name)
                shape = tuple(alloc.tensor_shape)
                dtype = _mybir.dt.np(alloc.dtype)
                out_avals.append(jax.core.ShapedArray(shape, dtype))
                zero_outs.append(np.zeros(shape, dtype))
        self.in_names, self.out_names = in_names, out_names
        self.out_avals, self.zero_outs = out_avals, zero_outs
        n_params, n_outs = len(in_names), len(out_names)
        in_names_full = in_names + out_names
        if pname is not None:
            in_names_full.append(pname)

        def _body(*args):
            operands = list(args)
            if pname is not None:
                operands.append(bass2jax.partition_id_tensor())
            outs = bass2jax._bass_exec_p.bind(
                *operands, out_avals=tuple(out_avals),
                in_names=tuple(in_names_full), out_names=tuple(out_names),
                lowering_input_output_aliases=(), sim_require_finite=True,
                sim_require_nnan=True, nc=nc)
            return tuple(outs)

        devices = jax.devices()[:NCORES]
        assert len(devices) == NCORES
        mesh = Mesh(np.asarray(devices), ("core",))
        self.sharding = NamedSharding(mesh, PartitionSpec("core"))
        donate = tuple(range(n_params, n_params + n_outs))
        self.sharded = jax.jit(
            _shard_map(_body, mesh,
                       (PartitionSpec("core"),) * (n_params + n_outs),
                       (PartitionSpec("core"),) * n_outs),
            donate_argnums=donate, keep_unused=True)

    def put_inputs(self, in_maps):
        """Concat per-core inputs on axis 0 and transfer to the devices."""
        concat = [np.concatenate([np.asarray(m[nm]) for m in in_maps], axis=0)
                  for nm in self.in_names]
        dev = self.jax.device_put(concat, [self.sharding] * len(concat))
        self.jax.block_until_ready(dev)
        return dev

    def put_outbufs(self):
        """Fresh donated output buffers (kernel fully overwrites them)."""
        concat = [np.zeros((NCORES * z.shape[0], *z.shape[1:]), z.dtype)
                  for z in self.zero_outs]
        dev = self.jax.device_put(concat, [self.sharding] * len(concat))
        self.jax.block_until_ready(dev)
        return dev

    def execute(self, dev_in, dev_outs):
        """One kernel execution; returns device output arrays (async)."""
        return self.sharded(*dev_in, *dev_outs)

    def fetch(self, out_arrs):
        """Device->host; returns per-core result dicts."""
        host = [np.asarray(o) for o in out_arrs]
        return [{nm: host[i].reshape(NCORES, *self.out_avals[i].shape)[c]
                 for i, nm in enumerate(self.out_names)}
                for c in range(NCORES)]


def _get_runner():
    if "runner" not in _CACHE:
        _CACHE["runner"] = _PjrtRunner(_get_program())
    return _CACHE["runner"]


def _shared_inputs(W_ih0, W_hh0, b0, W_hr0, W_ih1, W_hh1, b1, W_hr1,
                   h_init, c_init, t_steps=T):
    perm = _gate_perm()
    Wi0, Wh0, b0p = W_ih0[perm], W_hh0[perm], b0[perm]
    Wi1, Wh1, b1p = W_ih1[perm], W_hh1[perm], b1[perm]

    sh = {
        "wa": _bf(Wi0[:, 0:128].T),
        "wxb": _bf(np.concatenate([Wi0[:, 128:135].T, b0p[None, :]], axis=0)),
        "wh0f": _bf((Wh0 @ W_hr0).T),
        "wh0i": _bf(Wh0.T),
        "wA1": _bf((Wi1 @ W_hr0).T),
        "wB1": _bf((Wh1 @ W_hr1).T),
        "wh1i": _bf(Wh1.T),
        "b1r": _bf(b1p[None, :]),
        "b1c": _f32(b1p.reshape(4, 128).T),
        "wr1": _bf(W_hr1.T),
        "h0i": _bf(np.broadcast_to(h_init[0].reshape(PJ, 1), (PJ, B))),
        "h1i": _bf(np.broadcast_to(h_init[1].reshape(PJ, 1), (PJ, B))),
        "one": _bf(np.ones((1, B), np.float32)),
        "c0i": _bf(np.broadcast_to(c_init[0].reshape(H, 1), (H, B))),
        "c1i": _bf(np.broadcast_to(c_init[1].reshape(H, 1), (H, B))),
        "pri": _f32(np.concatenate([_length_priors_np()[:t_steps].T] * 2,
                                   axis=0)),
        "iot": _bf(np.broadcast_to(np.arange(64, dtype=np.float32)[None, :], (128, 64))),
        "idn": _f32(np.eye(128, dtype=np.float32)),
        "epb": _f32(np.full((128, 1), EPS, np.float32)),
    }
    return sh


def _core_inputs(inputs_seq, tokens, core, t_steps=T):
    xs = inputs_seq[:t_steps, core * B:(core + 1) * B, :]       # [T, 512, 135]
    xT = np.ascontiguousarray(np.transpose(xs, (0, 2, 1)))      # [T, 135, 512]
    toks = tokens[core * B:(core + 1) * B, :t_steps]            # [512, T]
    tokl = np.ascontiguousarray(
        np.transpose(toks.reshape(G, 128, t_steps), (1, 0, 2)).reshape(128, G * t_steps))
    return {
        "xa": _bf(xT[:, 0:128, :]),
        "xb": _bf(xT[:, 128:135, :]),
        "tok": _bf(tokl),
    }


def _make_in_maps(inputs_seq, W_ih0, W_hh0, b0, W_hr0, W_ih1, W_hh1, b1,
                  W_hr1, h_init, c_init, tokens):
    inputs_seq = np.asarray(inputs_seq, dtype=np.float32)
    tokens_np = np.asarray(tokens)
    sh = _shared_inputs(np.asarray(W_ih0, np.float32), np.asarray(W_hh0, np.float32),
                        np.asarray(b0, np.float32), np.asarray(W_hr0, np.float32),
                        np.asarray(W_ih1, np.float32), np.asarray(W_hh1, np.float32),
                        np.asarray(b1, np.float32), np.asarray(W_hr1, np.float32),
                        np.asarray(h_init, np.float32), np.asarray(c_init, np.float32))
    in_maps = []
    for core in range(NCORES):
        m = dict(sh)
        m.update(_core_inputs(inputs_seq, tokens_np, core))
        in_maps.append(m)
    return in_maps


def _unshard(results):
    ents = np.empty((NB, T), np.float32)
    lps = np.empty((NB, T), np.float32)
    for core in range(NCORES):
        r = results[core]
        e = r["ents"].reshape(128, G, T).transpose(1, 0, 2).reshape(B, T)
        l = r["lps"].reshape(128, G, T).transpose(1, 0, 2).reshape(B, T)
        ents[core * B:(core + 1) * B] = e
        lps[core * B:(core + 1) * B] = l
    return ents, lps


def timed_execs(n_reps):
    """Run the already-staged kernel n_reps times back-to-back on the
    device-resident inputs and return the total wall-clock ns.  Executions
    serialize on the NeuronCores (each rep's donated output buffers are the
    previous rep's outputs), so wall/n_reps upper-bounds the per-execution
    device time; differencing two rep counts cancels the constant dispatch
    round-trip latency of the axon tunnel."""
    import time as _time
    runner = _CACHE["runner"]
    dev_in = _CACHE["dev_in"]
    outs = runner.execute(dev_in, runner.put_outbufs())  # warm/stage
    runner.jax.block_until_ready(outs)
    t0 = _time.perf_counter()
    for _ in range(n_reps):
        outs = runner.execute(dev_in, outs)
    runner.jax.block_until_ready(outs)
    return (_time.perf_counter() - t0) * 1e9


def kernel(inputs_seq, W_ih0, W_hh0, b0, W_hr0, W_ih1, W_hh1, b1, W_hr1,
           h_init, c_init, tokens, _trace=False):
    in_maps = _make_in_maps(inputs_seq, W_ih0, W_hh0, b0, W_hr0, W_ih1,
                            W_hh1, b1, W_hr1, h_init, c_init, tokens)
    import time as _time
    try:
        runner = _get_runner()
        dev_in = runner.put_inputs(in_maps)
        _CACHE["dev_in"] = dev_in
        _t0 = _time.perf_counter()
        out_arrs = runner.execute(dev_in, runner.put_outbufs())
        runner.jax.block_until_ready(out_arrs)
        _CACHE["exec_wall_ns"] = (_time.perf_counter() - _t0) * 1e9
        results = runner.fetch(out_arrs)
    except Exception:
        # fall back to the stock SPMD path
        nc = _get_program()
        _t0 = _time.perf_counter()
        res = run_bass_kernel_spmd(nc, in_maps, core_ids=list(range(NCORES)))
        _CACHE["exec_wall_ns"] = (_time.perf_counter() - _t0) * 1e9
        results = res.results
    return _unshard(results)



# revision 51
# speedup vs baseline: 1.2623x; 1.1091x over previous
"""Trainium2 Bass kernel for DSOAgent sampling (2-layer projected LSTM decode).

Math per step t (batch n, per core n=512):
  L0: gates = W_ih0 @ x_t + W_hh0 @ h0 + b0 ; c0' = sig(f)*c0 + sig(i)*tanh(g)
      h0' = (sig(o)*tanh(c0')) @ W_hr0.T
  L1: same with h0' as input -> h1'
  logits = h1' + prior[t];  p = softmax(logits)+eps (renorm ~1)
  ent[:,t] = -sum p*log(p);  lp[:,t] = log(p)[tokens[:,t]]

Sharding: pure data parallel, batch 4096 -> 8 cores x 512.

Device layout: feature-major [feat_part, batch_free] for the recurrence;
softmax done batch-major after a PE transpose.  Gate banks are ordered
(f, i, o, g) so one batched Sigmoid covers f,i,o.

Performance structure (what made it fast):
- Projection fusion on host: W_hh0@W_hr0, W_ih1@W_hr0, W_hh1@W_hr1 are
  pre-multiplied, so the recurrent state is the 128-dim pre-projection
  activation (hp) and no psum->SBUF state copies sit on the loop.  Step 0
  uses the unfused 64-dim form to consume h_init/c_init exactly.
- Biases never cost matmuls: layer 0's rides the ones-row in the xb
  moving chunk; layer 1's is applied as the per-partition ACT bias of
  per-gate full-width sigmoid/tanh ops (dropping the 4 K=1 bias matmuls
  per step cut ~1000 instructions and ~0.4ms of real-HW marginal time,
  and lets the cell update start as soon as the f-gate sigmoid lands).
- Each layer's gates use a 3-bank f/i/o psum pool + 1-bank g pool; the
  hr1/transpose/backlog scratch is time-multiplexed into the consumed
  g bank, so all 8 psum banks are productive and the sigma-critical banks
  recycle immediately after the Sigmoid reads them.
- The whole recurrence (matmuls, sigma via 3D strided APs, tanh, cell
  update, h_pre) is split into two independent 256-column batch halves
  that pipeline through PE->ACT->DVE; matmul groups are uniform
  half-width so psum accumulation-group tracking stays valid.
- Recurrence-chain ops are priority-boosted (tc.high_priority; L0 above
  L1) so the scheduler never wedges off-chain work into the serial loop.
- The softmax tail is computed per 32-step block WITHOUT materializing p
  or log(p): ent = lnZ - sum(x*e^x)/Z and lp_tok = ln(p_tok + eps) with
  p_tok = (sum_v onehot(tok)*e^x)/Z gathered by an is_equal mask and a
  3D tensor_reduce.  This replaces the per-(step,group) tiny
  scalar_tensor_tensor ops with a few wide DVE ops (drip-issued in
  8-step chunks across the next block) and shrinks the per-block ACT
  phase to three
  back-to-back ops (Exp, Ln(Z), Ln(p_tok)) under one exp/ln table phase
  (priority-boosted so no sigmoid wedges in and thrashes the table).
  bf16 e^x underflow at x<-87 reproduces the 1e-10 log clamp exactly.
- Full-width (512-col) matmuls and 4-step-batched xa/xb DMAs: the sim
  says half-width pipelines marginally better, but real-HW marginal
  per-exec time drops ~0.25ms from the halved PE instruction count
  (per-exec instruction fetch dominates the sim-vs-HW gap).
- bf16 matmul operands and cell state (forget-gate decay bounds drift;
  validated vs fp32), fp32 psum/logits so entropy/log-prob errors stay
  ~1e-3 relative (tolerance 2e-2).

Measurement: NTFF profiling is unavailable under axon here, so the "HW
exec time" is the marginal wall-clock per execution: back-to-back reps
on device-resident inputs (donated-output chaining serializes reps on
the cores), differenced between two rep counts to cancel the constant
dispatch round-trip.  Upper-bounds true device time (includes per-exec
NEFF launch overhead, ~0.8ms of the ~2ms).
"""

import os
from contextlib import ExitStack

import ml_dtypes
import numpy as np

import concourse.bass as bass
import concourse.tile as tile
from concourse import bacc, mybir
from concourse.bass_utils import run_bass_kernel_spmd
from concourse.tile_rust import add_dep_helper as _add_dep_raw


def add_dep_helper(frm, to, sync=True, reason=""):
    _add_dep_raw(getattr(frm, "ins", frm), getattr(to, "ins", to),
                 sync=sync, reason=reason)

F32 = mybir.dt.float32
BF16 = mybir.dt.bfloat16
AF = mybir.ActivationFunctionType
OP = mybir.AluOpType

T = 128          # decode steps
NB = 4096        # total batch
IN = 135         # input feature size
H = 128          # LSTM hidden
PJ = 64          # proj size / vocab
NCORES = 8
B = NB // NCORES  # per-core batch = 512
G = B // 128      # batch groups of 128 partitions = 4
KBLK = int(os.environ.get("K_KBLK", "32"))  # steps per softmax block
NBLK = T // KBLK
EPS = 1e-10

# batch-split factors per engine (env-settable for TimelineSim A/B runs);
# the recurrence is issued as independent per-slice chains that pipeline.
SPLIT_MM = int(os.environ.get("K_SPLIT_MM", "1"))
SPLIT_ACT = int(os.environ.get("K_SPLIT_ACT", "2"))
SPLIT_DVE = int(os.environ.get("K_SPLIT_DVE", "2"))
SPLIT_ACT1 = int(os.environ.get("K_SPLIT_ACT1", str(SPLIT_ACT)))
SPLIT_DVE1 = int(os.environ.get("K_SPLIT_DVE1", str(SPLIT_DVE)))
B1ACT = os.environ.get("K_B1ACT", "1") == "1"    # L1 bias via ACT bias operand
L0GATE = os.environ.get("K_L0GATE", "0") == "1"  # per-gate full-width L0 sigmoid
T2 = os.environ.get("K_T2", "0") == "1"          # 2-step packed logits transpose
HPF = os.environ.get("K_HPF", "0") == "1"        # full-width end-of-chain hp mults


def _slices(n):
    w = 512 // n
    return [slice(i * w, (i + 1) * w) for i in range(n)]

# PyTorch gate order i,f,g,o; we reorder rows to (f, i, o, g) so the three
# sigmoid gates occupy adjacent psum banks.
def _gate_perm():
    i = np.arange(0, H)
    f = np.arange(H, 2 * H)
    g = np.arange(2 * H, 3 * H)
    o = np.arange(3 * H, 4 * H)
    return np.concatenate([f, i, o, g])


def _bf(x):
    return np.ascontiguousarray(x.astype(ml_dtypes.bfloat16))


def _f32(x):
    return np.ascontiguousarray(x.astype(np.float32))


def _length_priors_np():
    t = np.arange(T, dtype=np.float32)
    idx = np.arange(PJ)
    zero_mask = ((idx >= 0) & (idx < 32)).astype(np.float32)
    two_mask = ((idx >= 48) & (idx < 64)).astype(np.float32)
    pen_short = np.where(t < 64.0, -((64.0 - t) ** 2) / 16.0, 0.0).astype(np.float32)
    pen_long = np.where(t > 64.0, -((t - 64.0) ** 2) / 16.0, 0.0).astype(np.float32)
    return pen_short[:, None] * zero_mask[None, :] + pen_long[:, None] * two_mask[None, :]


def build_program(t_steps=T, kblk=KBLK):
    """Build and compile the single-core Bass program (same program runs on
    all 8 cores, SPMD over the batch)."""
    nblk = t_steps // kblk
    nc = bacc.Bacc(
        "TRN2",
        target_bir_lowering=False,
        debug=False,
        enable_asserts=False,
        num_devices=1,
    )

    # ---- DRAM I/O ----
    d_xa = nc.dram_tensor("xa", [t_steps, 128, B], BF16, kind="ExternalInput").ap()
    d_xb = nc.dram_tensor("xb", [t_steps, 7, B], BF16, kind="ExternalInput").ap()
    d_wa = nc.dram_tensor("wa", [128, 512], BF16, kind="ExternalInput").ap()
    d_wxb = nc.dram_tensor("wxb", [8, 512], BF16, kind="ExternalInput").ap()
    d_wh0f = nc.dram_tensor("wh0f", [128, 512], BF16, kind="ExternalInput").ap()
    d_wh0i = nc.dram_tensor("wh0i", [64, 512], BF16, kind="ExternalInput").ap()
    d_wA1 = nc.dram_tensor("wA1", [128, 512], BF16, kind="ExternalInput").ap()
    d_wB1 = nc.dram_tensor("wB1", [128, 512], BF16, kind="ExternalInput").ap()
    d_wh1i = nc.dram_tensor("wh1i", [64, 512], BF16, kind="ExternalInput").ap()
    d_b1r = nc.dram_tensor("b1r", [1, 512], BF16, kind="ExternalInput").ap()
    d_b1c = nc.dram_tensor("b1c", [128, 4], F32, kind="ExternalInput").ap()
    d_wr1 = nc.dram_tensor("wr1", [128, 64], BF16, kind="ExternalInput").ap()
    d_h0i = nc.dram_tensor("h0i", [64, B], BF16, kind="ExternalInput").ap()
    d_h1i = nc.dram_tensor("h1i", [64, B], BF16, kind="ExternalInput").ap()
    d_c0i = nc.dram_tensor("c0i", [128, B], BF16, kind="ExternalInput").ap()
    d_c1i = nc.dram_tensor("c1i", [128, B], BF16, kind="ExternalInput").ap()
    d_tok = nc.dram_tensor("tok", [128, G * t_steps], BF16, kind="ExternalInput").ap()
    d_pri = nc.dram_tensor("pri", [128, t_steps], F32, kind="ExternalInput").ap()
    d_iot = nc.dram_tensor("iot", [128, 64], BF16, kind="ExternalInput").ap()
    d_idn = nc.dram_tensor("idn", [128, 128], F32, kind="ExternalInput").ap()
    d_epb = nc.dram_tensor("epb", [128, 1], F32, kind="ExternalInput").ap()
    d_one = nc.dram_tensor("one", [1, B], BF16, kind="ExternalInput").ap()
    d_ent = nc.dram_tensor("ents", [128, G * t_steps], F32, kind="ExternalOutput").ap()
    d_lp = nc.dram_tensor("lps", [128, G * t_steps], F32, kind="ExternalOutput").ap()

    with tile.TileContext(nc) as tc, ExitStack() as ctx:
        _build_tile(ctx, tc, t_steps, kblk, nblk, dict(
            xa=d_xa, xb=d_xb, wa=d_wa, wxb=d_wxb, wh0f=d_wh0f, wh0i=d_wh0i,
            wA1=d_wA1, wB1=d_wB1, wh1i=d_wh1i, b1r=d_b1r, b1c=d_b1c, wr1=d_wr1, h0i=d_h0i, h1i=d_h1i, c0i=d_c0i, c1i=d_c1i, one=d_one,
            tok=d_tok, pri=d_pri, iot=d_iot, idn=d_idn, epb=d_epb, ent=d_ent, lp=d_lp,
        ))

    nc.compile()
    return nc


def _build_tile(ctx, tc, t_steps, kblk, nblk, io):
    nc = tc.nc

    cst = ctx.enter_context(tc.tile_pool(name="cst", bufs=1))
    st = ctx.enter_context(tc.tile_pool(name="st", bufs=1))
    wk = ctx.enter_context(tc.tile_pool(name="wk", bufs=3))
    wkx = ctx.enter_context(tc.tile_pool(name="wkx", bufs=3))
    pgL0f = ctx.enter_context(tc.tile_pool(name="pgL0f", bufs=1, space="PSUM"))
    pgL0g = ctx.enter_context(tc.tile_pool(name="pgL0g", bufs=1, space="PSUM"))
    pgL1f = ctx.enter_context(tc.tile_pool(name="pgL1f", bufs=1, space="PSUM"))
    pgL1g = ctx.enter_context(tc.tile_pool(name="pgL1g", bufs=1, space="PSUM"))

    def load_const(name, shape, dt):
        t_ = cst.tile(shape, dt, tag=name)
        nc.sync.dma_start(t_[:], io[name][:])
        return t_

    wa = load_const("wa", [128, 512], BF16)
    wxb = load_const("wxb", [8, 512], BF16)
    wh0f = load_const("wh0f", [128, 512], BF16)
    wh0i = load_const("wh0i", [64, 512], BF16)
    wA1 = load_const("wA1", [128, 512], BF16)
    wB1 = load_const("wB1", [128, 512], BF16)
    wh1i = load_const("wh1i", [64, 512], BF16)
    b1r = load_const("b1r", [1, 512], BF16)
    b1c = load_const("b1c", [128, 4], F32)
    one_t = load_const("one", [1, B], BF16)
    wr1 = load_const("wr1", [128, 64], BF16)
    tok = load_const("tok", [128, G * t_steps], BF16)
    pri = load_const("pri", [128, t_steps], F32)
    iot = load_const("iot", [128, 64], BF16)
    idn = load_const("idn", [128, 128], F32)
    epb = load_const("epb", [128, 1], F32)

    # persistent state (double-buffered across steps)
    # sxb: [xb(7); ones(1)] input chunk; hp0: layer-0 pre-projection state;
    # h1: [h1(64); ones(1)] layer-1 projected state; h0i/h1i initial h states
    # xa/xb stream in XB4-step batches (one DMA per tensor per XB4 steps)
    XB4 = int(os.environ.get("K_XB4", "4"))
    sxb = [st.tile([8, XB4 * B], BF16, tag=f"sxb_{k}", name=f"sxb_{k}")
           for k in range(2)]
    hp0s = [st.tile([128, B], BF16, tag=f"hp0s_{k}", name=f"hp0s_{k}") for k in range(2)]
    hp1s = [st.tile([128, B], BF16, tag=f"hp1s_{k}", name=f"hp1s_{k}") for k in range(2)]
    h0i = st.tile([64, B], BF16, tag="h0i", name="h0i")
    h1i = st.tile([64, B], BF16, tag="h1i", name="h1i")
    c0 = st.tile([128, B], BF16, tag="c0", name="c0")
    c1 = st.tile([128, B], BF16, tag="c1", name="c1")
    for k in range(2):
        for j in range(XB4):
            nc.sync.dma_start(sxb[k][7:8, j * B:(j + 1) * B], io["one"][:])
    nc.sync.dma_start(h0i[:], io["h0i"][:])
    nc.sync.dma_start(h1i[:], io["h1i"][:])
    nc.sync.dma_start(c0[:], io["c0i"][:])
    nc.sync.dma_start(c1[:], io["c1i"][:])

    # softmax block buffers.  Per block (16 steps x 4 groups x 64 vocab):
    #   e = exp(x); Z = sum_v e; ent = lnZ - sum_v(x*e)/Z
    #   lp_tok = ln(exp(x_tok - lnZ) + eps)  (x_tok via one-hot gather-reduce)
    backlog = [st.tile([128, kblk * 256], F32, tag=f"bl_{k}", name=f"bl_{k}") for k in range(2)]
    e_blk = st.tile([128, kblk * 256], BF16, tag="e_blk", name="e_blk")
    q_blk = st.tile([128, kblk * 256], BF16, tag="q_blk", name="q_blk")
    eq_blk = st.tile([128, kblk * 256], BF16, tag="eq_blk", name="eq_blk")
    ql_blk = q_blk  # shared scratch: s2 consumes q before ql writes (drip order)
    zs = st.tile([128, kblk * G], F32, tag="zs", name="zs")
    rz = st.tile([128, kblk * G], F32, tag="rz", name="rz")
    s2n = st.tile([128, kblk * G], F32, tag="s2n", name="s2n")
    ptk = st.tile([128, kblk * G], F32, tag="ptk", name="ptk")
    lnz = st.tile([128, kblk * G], F32, tag="lnz", name="lnz")
    pt2 = st.tile([128, kblk * G], F32, tag="pt2", name="pt2")
    se = st.tile([128, kblk * G], F32, tag="se", name="se")
    ent_o = st.tile([128, G * t_steps], F32, tag="ent_o", name="ent_o")
    lp_o = st.tile([128, G * t_steps], F32, tag="lp_o", name="lp_o")

    last_act = [None]   # last recurrence ACT op of current block
    deferred = []       # phase tail ops, drip-issued into the next block

    def act(*a, **k):
        op = nc.scalar.activation(*a, **k)
        last_act[0] = op
        return op

    def _sg4(tile_, off=0):
        """[128, kblk*256] -> [128, s, g, v] 4D view."""
        return tile_[:].rearrange("p (s g v) -> p s g v", g=G, v=64)

    def _sg(tile_):
        """[128, kblk*G] -> [128, s, g] view (s-major, matching reduces)."""
        return tile_[:].rearrange("p (s g) -> p s g", g=G)

    def _cols(out_tile, blk):
        """[128, s, g] strided view into out_tile's (g*T + blk*kblk + s) cols."""
        v = out_tile[:].rearrange("p (g t) -> p g t", t=t_steps)
        v = v[:, :, blk * kblk:(blk + 1) * kblk]
        return v.rearrange("p g s -> p s g")

    def _tok4(blk, s0, ns):
        """tokens for steps [blk*kblk+s0, +ns) as [128, ns, g, v(bcast)]."""
        v = tok[:].rearrange("p (g t) -> p g t", t=t_steps)
        v = v[:, :, blk * kblk + s0:blk * kblk + s0 + ns]
        v = v.rearrange("p g s -> p s g")
        return v.rearrange("p s (g o) -> p s g o", o=1).broadcast_to(
            [128, ns, G, 64])

    def softmax_phase(blk):
        """Emit the exp-table ACT group: Exp for block blk plus the two Ln
        ops finishing block blk-1 (all inputs pre-computed by the drip, so
        the three ops run back-to-back under one table phase)."""
        pbk = blk % 2
        with tc.high_priority(70000):
            exp_op = nc.scalar.activation(e_blk[:], backlog[pbk][:], AF.Exp)
            if os.environ.get("K_EXP_NODEP", "0") != "1":
                add_dep_helper(exp_op, last_act[0], sync=False,
                               reason="exp after recurrence ACT of block")
            if blk > 0:
                # lnZ of block blk-1 (zs ready from last block's drip)
                op = nc.scalar.activation(lnz[:], zs[:], AF.Ln)
                add_dep_helper(op, exp_op, sync=False, reason="lnz after exp")
                # lp(blk-1) = ln(p_tok + eps); p_tok = (sum eq*e)/Z from drip
                op2 = nc.scalar.activation(_cols(lp_o, blk - 1), _sg(pt2),
                                           AF.Ln, bias=epb[:])
                add_dep_helper(op2, op, sync=False, reason="lp after lnz")
        _emit_drip(blk)

    def _emit_drip(blk):
        """Deferred DVE work: ent combine for blk-1, then reduces for blk,
        split into s-halves so no single drip op overruns a step's DVE
        slack."""
        pbk = blk % 2
        nch = max(1, kblk // 8)                # 8-step drip chunks
        hw_, hc = kblk // nch, kblk // nch * 256
        hg = kblk // nch * G

        def h4(tile_, hs):
            return tile_[:, hs * hc:(hs + 1) * hc].rearrange(
                "p (s g v) -> p s g v", g=G, v=64)

        def hsg(tile_, hs):
            return tile_[:, hs * hg:(hs + 1) * hg].rearrange(
                "p (s g) -> p s g", g=G)

        if blk > 0:
            def _ent(blk=blk):
                # ent(blk-1) = lnz + (-s2)/Z  (lnz from this phase's ACT)
                nc.vector.tensor_tensor(se[:], s2n[:], rz[:], OP.mult)
                nc.vector.tensor_tensor(_cols(ent_o, blk - 1), _sg(lnz),
                                        _sg(se), OP.add)
            deferred.append(_ent)

        for hs in range(nch):
            def _z(hs=hs):
                nc.vector.tensor_reduce(hsg(zs, hs), h4(e_blk, hs),
                                        axis=mybir.AxisListType.X, op=OP.add)
                if hs == nch - 1:
                    nc.vector.reciprocal(rz[:], zs[:])
            deferred.append(_z)
        for hs in range(nch):
            def _q(hs=hs, pbk=pbk):
                nc.vector.tensor_tensor(
                    q_blk[:, hs * hc:(hs + 1) * hc],
                    backlog[pbk][:, hs * hc:(hs + 1) * hc],
                    e_blk[:, hs * hc:(hs + 1) * hc], OP.mult)
            deferred.append(_q)
        for hs in range(nch):
            def _s2(hs=hs):
                nc.vector.tensor_reduce(hsg(s2n, hs), h4(q_blk, hs),
                                        axis=mybir.AxisListType.X, op=OP.add,
                                        negate=True)
            deferred.append(_s2)
        for hs in range(nch):
            def _eq(hs=hs, blk=blk):
                iot4 = iot[:].rearrange("p (a b v) -> p a b v",
                                        a=1, b=1).broadcast_to(
                    [128, hw_, G, 64])
                tok4 = _tok4(blk, hs * hw_, hw_)
                nc.vector.tensor_tensor(h4(eq_blk, hs), iot4, tok4,
                                        OP.is_equal)
            deferred.append(_eq)
        for hs in range(nch):
            def _ql(hs=hs):
                nc.vector.tensor_tensor(
                    ql_blk[:, hs * hc:(hs + 1) * hc],
                    eq_blk[:, hs * hc:(hs + 1) * hc],
                    e_blk[:, hs * hc:(hs + 1) * hc], OP.mult)
            deferred.append(_ql)
        for hs in range(nch):
            def _ptk(hs=hs):
                nc.vector.tensor_reduce(hsg(ptk, hs), h4(ql_blk, hs),
                                        axis=mybir.AxisListType.X, op=OP.add)
            deferred.append(_ptk)

        def _pt2():
            nc.vector.tensor_tensor(pt2[:], ptk[:], rz[:], OP.mult)
        deferred.append(_pt2)

    def final_phase(blk):
        """Finish block blk's outputs at the end of the program."""
        op = nc.scalar.activation(lnz[:], zs[:], AF.Ln)
        add_dep_helper(op, last_act[0], sync=False, reason="final lnz")
        op2 = nc.scalar.activation(_cols(lp_o, blk), _sg(pt2), AF.Ln,
                                   bias=epb[:])
        add_dep_helper(op2, op, sync=False, reason="final lp")
        nc.vector.tensor_tensor(se[:], s2n[:], rz[:], OP.mult)
        nc.vector.tensor_tensor(_cols(ent_o, blk), _sg(lnz), _sg(se), OP.add)

    for t in range(t_steps):
        p_, pn = t % 2, (t + 1) % 2
        blk, s_in = t // kblk, t % kblk

        # input DMAs (batched: one DMA covers 4 steps)
        g4, s4 = t // XB4, t % XB4
        if s4 == 0:
            xa4_cur = wkx.tile([128, XB4 * B], BF16, tag="xa", name="xa")
            nc.sync.dma_start(
                xa4_cur[:].rearrange("p (t b) -> p t b", t=XB4),
                io["xa"][t:t + XB4].rearrange("t p b -> p t b"))
            nc.sync.dma_start(
                sxb[g4 % 2][0:7, :].rearrange("p (t b) -> p t b", t=XB4),
                io["xb"][t:t + XB4].rearrange("t p b -> p t b"))
        xa_tile, sxb_tile, xoff = xa4_cur, sxb[g4 % 2], s4 * B

        mm_slices = _slices(SPLIT_MM)
        act_slices = _slices(SPLIT_ACT)
        dve_slices = _slices(SPLIT_DVE)

        def gsl(g, sl):
            return slice(g * 512 + sl.start, g * 512 + sl.stop)

        # ---- layer 0 gates: psum banks (f, i, o) + (g) ----
        gf = pgL0f.tile([128, 1536], F32, tag="gL0f", name="gL0f")
        gg = pgL0g.tile([128, 512], F32, tag="gL0g", name="gL0g")
        with tc.high_priority(60000):
            for hsl in mm_slices:
                for m in range(4):
                    out = gf[:, m * 512:(m + 1) * 512] if m < 3 else gg[:]
                    msl = slice(m * 128, (m + 1) * 128)
                    nc.tensor.matmul(
                        out[:, hsl], wa[:, msl],
                        xa_tile[:, xoff + hsl.start:xoff + hsl.stop],
                        start=True, stop=False)
                    nc.tensor.matmul(
                        out[:, hsl], wxb[:, msl],
                        sxb_tile[:, xoff + hsl.start:xoff + hsl.stop],
                        start=False, stop=False)
                    if t == 0:
                        nc.tensor.matmul(out[:, hsl], wh0i[:, msl], h0i[:, hsl],
                                         start=False, stop=True)
                    else:
                        nc.tensor.matmul(out[:, hsl], wh0f[:, msl],
                                         hp0s[p_][:, hsl], start=False, stop=True)

        sfio = wk.tile([128, 1536], BF16, tag="sfio", name="sfio")
        with tc.high_priority(60000):
            gf3 = gf[:].rearrange("p (b n) -> p b n", n=512)
            sf3 = sfio[:].rearrange("p (b n) -> p b n", n=512)
            first_sig = None
            if L0GATE:
                for gi in range(3):
                    op = nc.scalar.activation(
                        sfio[:, gi * 512:(gi + 1) * 512],
                        gf[:, gi * 512:(gi + 1) * 512], AF.Sigmoid)
                    if first_sig is None:
                        first_sig = op
            else:
                for sl in act_slices:
                    op = nc.scalar.activation(sf3[:, :, sl], gf3[:, :, sl],
                                              AF.Sigmoid)
                    if first_sig is None:
                        first_sig = op
        # no hard exp->sigma gate: letting the scheduler interleave the next
        # block's recurrence through the ln/exp phase keeps PE warm; costs a
        # couple of extra ACT table loads per block (counted: 36 vs 24 total)
        # but nets faster overall.
        last_act[0] = first_sig
        tg = wk.tile([128, 512], BF16, tag="tg", name="tg")
        m0 = wk.tile([128, 512], BF16, tag="m0", name="m0")
        t1 = wk.tile([128, 512], BF16, tag="t1", name="t1")
        tc0 = wk.tile([128, 512], BF16, tag="tc0", name="tc0")
        hp0 = hp0s[pn]
        with tc.high_priority(60000):
            for sl in act_slices:
                act(tg[:, sl], gg[:, sl], AF.Tanh)
            for sl in dve_slices:
                # t1 first: it needs only sig(f)+old c, so DVE starts while
                # ACT still runs tanh(g)
                nc.vector.tensor_tensor(
                    t1[:, sl], sfio[:, gsl(0, sl)], c0[:, sl], OP.mult)
                nc.vector.tensor_tensor(
                    m0[:, sl], sfio[:, gsl(1, sl)], tg[:, sl], OP.mult)
                nc.vector.tensor_tensor(c0[:, sl], m0[:, sl], t1[:, sl], OP.add)
            for sl in act_slices:
                act(tc0[:, sl], c0[:, sl], AF.Tanh)
            for sl in (_slices(1) if HPF else dve_slices):
                nc.vector.tensor_tensor(
                    hp0[:, sl], sfio[:, gsl(2, sl)], tc0[:, sl], OP.mult)

        # ---- layer 1 (input side fused with W_hr0) ----
        gf2 = pgL1f.tile([128, 1536], F32, tag="gL1f", name="gL1f")
        gg2 = pgL1g.tile([128, 512], F32, tag="gL1g", name="gL1g")
        with tc.high_priority(50000):
            for hsl in mm_slices:
                for m in ([3, 0, 1, 2] if os.environ.get("K_GFIRST", "0") == "1"
                          else range(4)):
                    out = gf2[:, m * 512:(m + 1) * 512] if m < 3 else gg2[:]
                    msl = slice(m * 128, (m + 1) * 128)
                    if not B1ACT:
                        nc.tensor.matmul(out[:, hsl], b1r[:, msl], one_t[:, hsl],
                                         start=True, stop=False)
                    if t == 0:
                        nc.tensor.matmul(out[:, hsl], wh1i[:, msl], h1i[:, hsl],
                                         start=B1ACT, stop=False)
                    else:
                        nc.tensor.matmul(out[:, hsl], wB1[:, msl],
                                         hp1s[p_][:, hsl], start=B1ACT, stop=False)
                    nc.tensor.matmul(out[:, hsl], wA1[:, msl], hp0[:, hsl],
                                     start=False, stop=True)

        sfio1 = wk.tile([128, 1536], BF16, tag="sfio1", name="sfio1")
        with tc.high_priority(50000):
            gf23 = gf2[:].rearrange("p (b n) -> p b n", n=512)
            sf13 = sfio1[:].rearrange("p (b n) -> p b n", n=512)
            tg1 = wk.tile([128, 512], BF16, tag="tg1", name="tg1")
            m1 = wk.tile([128, 512], BF16, tag="m1", name="m1")
            t11 = wk.tile([128, 512], BF16, tag="t11", name="t11")
            tc1 = wk.tile([128, 512], BF16, tag="tc1", name="tc1")
            hp1 = hp1s[pn]
            if B1ACT:
                # per-gate full-width sigmoids so each takes its own b1 bias;
                # tanh(g) takes the g-gate bias the same way
                for gi in range(3):
                    act(sfio1[:, gi * 512:(gi + 1) * 512],
                        gf2[:, gi * 512:(gi + 1) * 512], AF.Sigmoid,
                        bias=b1c[:, gi:gi + 1])
                act(tg1[:], gg2[:], AF.Tanh, bias=b1c[:, 3:4])
            else:
                for sl in _slices(SPLIT_ACT1):
                    if os.environ.get("K_GFIRST", "0") == "1":
                        act(tg1[:, sl], gg2[:, sl], AF.Tanh)
                        act(sf13[:, :, sl], gf23[:, :, sl], AF.Sigmoid)
                    else:
                        act(sf13[:, :, sl], gf23[:, :, sl], AF.Sigmoid)
                        act(tg1[:, sl], gg2[:, sl], AF.Tanh)
            for sl in _slices(SPLIT_DVE1):
                nc.vector.tensor_tensor(
                    t11[:, sl], sfio1[:, gsl(0, sl)], c1[:, sl], OP.mult)
                nc.vector.tensor_tensor(
                    m1[:, sl], sfio1[:, gsl(1, sl)], tg1[:, sl], OP.mult)
                nc.vector.tensor_tensor(c1[:, sl], m1[:, sl], t11[:, sl], OP.add)
            for sl in _slices(SPLIT_ACT1):
                act(tc1[:, sl], c1[:, sl], AF.Tanh)
            for sl in (_slices(1) if HPF else _slices(SPLIT_DVE1)):
                nc.vector.tensor_tensor(
                    hp1[:, sl], sfio1[:, gsl(2, sl)], tc1[:, sl], OP.mult)

        if T2:
            # 2-step packed logits path: even steps park h1'+prior in rows
            # 0:64 of a shared [128,512] SBUF tile, odd steps in rows 64:128
            # (hr1 writes psum partitions 64:128 directly so the prior-add
            # stays lane-aligned); the pair is transposed and copied to the
            # backlog once, halving transpose/copy instructions.
            po = (t % 2) * 64
            aps_ = gg2[po:po + 64, :]
            nc.tensor.matmul(aps_, wr1[:], hp1[:], start=True, stop=True,
                             skip_group_check=True)
            if t % 2 == 0:
                lgt2 = wk.tile([128, 512], F32, tag="lgt", name="lgt")
            nc.vector.tensor_scalar(lgt2[po:po + 64, :], aps_,
                                    pri[po:po + 64, t:t + 1], None, OP.add)
            if t % 2 == 1:
                bps = gg2[0:128, :]
                for g in range(G):
                    nc.tensor.matmul(
                        bps[:, g * 128:(g + 1) * 128],
                        lgt2[:, g * 128:(g + 1) * 128], idn[:],
                        is_transpose=True, skip_group_check=True)
                src_ = bps.rearrange("p (g s v) -> p s g v", s=2, v=64)
                dst_ = backlog[blk % 2][:, (s_in - 1) * 256:(s_in + 1) * 256]
                dst_ = dst_.rearrange("p (s g v) -> p s g v", g=G, v=64)
                nc.vector.tensor_copy(dst_, src_)
        else:
            aps_ = gg2[0:64, :]
            nc.tensor.matmul(aps_, wr1[:], hp1[:], start=True, stop=True,
                             skip_group_check=True)
            lgt = wk.tile([64, 512], F32, tag="lgt", name="lgt")
            nc.vector.tensor_scalar(lgt[:], aps_, pri[0:64, t:t + 1], None,
                                    OP.add)
            bps = gg2[0:128, 0:256]
            for g in range(G):
                nc.tensor.matmul(
                    bps[:, g * 64:(g + 1) * 64], lgt[:, g * 128:(g + 1) * 128],
                    idn[0:64, 0:64], is_transpose=True, skip_group_check=True)
            nc.vector.tensor_copy(
                backlog[blk % 2][:, s_in * 256:(s_in + 1) * 256], bps)

        # drip-issue deferred phase-tail ops (after the step body so they
        # rank below this step's tail ops in the scheduler's tie-breaks)
        if deferred:
            deferred.pop(0)()

        if s_in == kblk - 1:
            softmax_phase(blk)

    # final block: drain deferred, then finish its ent/lp outputs
    while deferred:
        deferred.pop(0)()
    final_phase(nblk - 1)

    nc.sync.dma_start(io["ent"][:], ent_o[:])
    nc.sync.dma_start(io["lp"][:], lp_o[:])


# ---------------------------------------------------------------------------
# host side
# ---------------------------------------------------------------------------

_CACHE = {}


def _get_program():
    if "nc" not in _CACHE:
        _CACHE["nc"] = build_program()
    return _CACHE["nc"]


class _PjrtRunner:
    """Executes the compiled Bass module on the 8 NeuronCores via PJRT with
    the input staging (host->device transfer) split out from execution, so
    device execution time can be measured separately from the axon-tunnel
    transfer cost.  Same lowering path run_bass_kernel_spmd takes under
    axon (bass2jax._bass_exec_p -> neuronx_cc_hook -> NEFF)."""

    def __init__(self, nc):
        import jax
        from jax.sharding import Mesh, PartitionSpec, NamedSharding
        try:
            from jax import shard_map
            def _shard_map(f, mesh, in_specs, out_specs):
                return shard_map(f, mesh=mesh, in_specs=in_specs,
                                 out_specs=out_specs, check_vma=False)
        except Exception:
            from jax.experimental.shard_map import shard_map
            def _shard_map(f, mesh, in_specs, out_specs):
                return shard_map(f, mesh=mesh, in_specs=in_specs,
                                 out_specs=out_specs, check_rep=False)
        from concourse import bass2jax, mybir as _mybir

        bass2jax.install_neuronx_cc_hook()
        self.jax = jax
        self.nc = nc
        pname = nc.partition_id_tensor.name if nc.partition_id_tensor else None
        in_names, out_names, out_avals, zero_outs = [], [], [], []
        for alloc in nc.m.functions[0].allocations:
            if not isinstance(alloc, _mybir.MemoryLocationSet):
                continue
            name = alloc.memorylocations[0].name
            if alloc.kind == "ExternalInput":
                if name != pname:
                    in_names.append(name)
            elif alloc.kind == "ExternalOutput":
                out_names.append(name)
                shape = tuple(alloc.tensor_shape)
                dtype = _mybir.dt.np(alloc.dtype)
                out_avals.append(jax.core.ShapedArray(shape, dtype))
                zero_outs.append(np.zeros(shape, dtype))
        self.in_names, self.out_names = in_names, out_names
        self.out_avals, self.zero_outs = out_avals, zero_outs
        n_params, n_outs = len(in_names), len(out_names)
        in_names_full = in_names + out_names
        if pname is not None:
            in_names_full.append(pname)

        def _body(*args):
            operands = list(args)
            if pname is not None:
                operands.append(bass2jax.partition_id_tensor())
            outs = bass2jax._bass_exec_p.bind(
                *operands, out_avals=tuple(out_avals),
                in_names=tuple(in_names_full), out_names=tuple(out_names),
                lowering_input_output_aliases=(), sim_require_finite=True,
                sim_require_nnan=True, nc=nc)
            return tuple(outs)

        devices = jax.devices()[:NCORES]
        assert len(devices) == NCORES
        mesh = Mesh(np.asarray(devices), ("core",))
        self.sharding = NamedSharding(mesh, PartitionSpec("core"))
        donate = tuple(range(n_params, n_params + n_outs))
        self.sharded = jax.jit(
            _shard_map(_body, mesh,
                       (PartitionSpec("core"),) * (n_params + n_outs),
                       (PartitionSpec("core"),) * n_outs),
            donate_argnums=donate, keep_unused=True)

    def put_inputs(self, in_maps):
        """Concat per-core inputs on axis 0 and transfer to the devices."""
        concat = [np.concatenate([np.asarray(m[nm]) for m in in_maps], axis=0)
                  for nm in self.in_names]
        dev = self.jax.device_put(concat, [self.sharding] * len(concat))
        self.jax.block_until_ready(dev)
        return dev

    def put_outbufs(self):
        """Fresh donated output buffers (kernel fully overwrites them)."""
        concat = [np.zeros((NCORES * z.shape[0], *z.shape[1:]), z.dtype)
                  for z in self.zero_outs]
        dev = self.jax.device_put(concat, [self.sharding] * len(concat))
        self.jax.block_until_ready(dev)
        return dev

    def execute(self, dev_in, dev_outs):
        """One kernel execution; returns device output arrays (async)."""
        return self.sharded(*dev_in, *dev_outs)

    def fetch(self, out_arrs):
        """Device->host; returns per-core result dicts."""
        host = [np.asarray(o) for o in out_arrs]
        return [{nm: host[i].reshape(NCORES, *self.out_avals[i].shape)[c]
                 for i, nm in enumerate(self.out_names)}
                for c in range(NCORES)]


def _get_runner():
    if "runner" not in _CACHE:
        _CACHE["runner"] = _PjrtRunner(_get_program())
    return _CACHE["runner"]


def _shared_inputs(W_ih0, W_hh0, b0, W_hr0, W_ih1, W_hh1, b1, W_hr1,
                   h_init, c_init, t_steps=T):
    perm = _gate_perm()
    Wi0, Wh0, b0p = W_ih0[perm], W_hh0[perm], b0[perm]
    Wi1, Wh1, b1p = W_ih1[perm], W_hh1[perm], b1[perm]

    sh = {
        "wa": _bf(Wi0[:, 0:128].T),
        "wxb": _bf(np.concatenate([Wi0[:, 128:135].T, b0p[None, :]], axis=0)),
        "wh0f": _bf((Wh0 @ W_hr0).T),
        "wh0i": _bf(Wh0.T),
        "wA1": _bf((Wi1 @ W_hr0).T),
        "wB1": _bf((Wh1 @ W_hr1).T),
        "wh1i": _bf(Wh1.T),
        "b1r": _bf(b1p[None, :]),
        "b1c": _f32(b1p.reshape(4, 128).T),
        "wr1": _bf(W_hr1.T),
        "h0i": _bf(np.broadcast_to(h_init[0].reshape(PJ, 1), (PJ, B))),
        "h1i": _bf(np.broadcast_to(h_init[1].reshape(PJ, 1), (PJ, B))),
        "one": _bf(np.ones((1, B), np.float32)),
        "c0i": _bf(np.broadcast_to(c_init[0].reshape(H, 1), (H, B))),
        "c1i": _bf(np.broadcast_to(c_init[1].reshape(H, 1), (H, B))),
        "pri": _f32(np.concatenate([_length_priors_np()[:t_steps].T] * 2,
                                   axis=0)),
        "iot": _bf(np.broadcast_to(np.arange(64, dtype=np.float32)[None, :], (128, 64))),
        "idn": _f32(np.eye(128, dtype=np.float32)),
        "epb": _f32(np.full((128, 1), EPS, np.float32)),
    }
    return sh


def _core_inputs(inputs_seq, tokens, core, t_steps=T):
    xs = inputs_seq[:t_steps, core * B:(core + 1) * B, :]       # [T, 512, 135]
    xT = np.ascontiguousarray(np.transpose(xs, (0, 2, 1)))      # [T, 135, 512]
    toks = tokens[core * B:(core + 1) * B, :t_steps]            # [512, T]
    tokl = np.ascontiguousarray(
        np.transpose(toks.reshape(G, 128, t_steps), (1, 0, 2)).reshape(128, G * t_steps))
    return {
        "xa": _bf(xT[:, 0:128, :]),
        "xb": _bf(xT[:, 128:135, :]),
        "tok": _bf(tokl),
    }


def _make_in_maps(inputs_seq, W_ih0, W_hh0, b0, W_hr0, W_ih1, W_hh1, b1,
                  W_hr1, h_init, c_init, tokens):
    inputs_seq = np.asarray(inputs_seq, dtype=np.float32)
    tokens_np = np.asarray(tokens)
    sh = _shared_inputs(np.asarray(W_ih0, np.float32), np.asarray(W_hh0, np.float32),
                        np.asarray(b0, np.float32), np.asarray(W_hr0, np.float32),
                        np.asarray(W_ih1, np.float32), np.asarray(W_hh1, np.float32),
                        np.asarray(b1, np.float32), np.asarray(W_hr1, np.float32),
                        np.asarray(h_init, np.float32), np.asarray(c_init, np.float32))
    in_maps = []
    for core in range(NCORES):
        m = dict(sh)
        m.update(_core_inputs(inputs_seq, tokens_np, core))
        in_maps.append(m)
    return in_maps


def _unshard(results):
    ents = np.empty((NB, T), np.float32)
    lps = np.empty((NB, T), np.float32)
    for core in range(NCORES):
        r = results[core]
        e = r["ents"].reshape(128, G, T).transpose(1, 0, 2).reshape(B, T)
        l = r["lps"].reshape(128, G, T).transpose(1, 0, 2).reshape(B, T)
        ents[core * B:(core + 1) * B] = e
        lps[core * B:(core + 1) * B] = l
    return ents, lps


def timed_execs(n_reps):
    """Run the already-staged kernel n_reps times back-to-back on the
    device-resident inputs and return the total wall-clock ns.  Executions
    serialize on the NeuronCores (each rep's donated output buffers are the
    previous rep's outputs), so wall/n_reps upper-bounds the per-execution
    device time; differencing two rep counts cancels the constant dispatch
    round-trip latency of the axon tunnel."""
    import time as _time
    runner = _CACHE["runner"]
    dev_in = _CACHE["dev_in"]
    outs = runner.execute(dev_in, runner.put_outbufs())  # warm/stage
    runner.jax.block_until_ready(outs)
    t0 = _time.perf_counter()
    for _ in range(n_reps):
        outs = runner.execute(dev_in, outs)
    runner.jax.block_until_ready(outs)
    return (_time.perf_counter() - t0) * 1e9


def kernel(inputs_seq, W_ih0, W_hh0, b0, W_hr0, W_ih1, W_hh1, b1, W_hr1,
           h_init, c_init, tokens, _trace=False):
    in_maps = _make_in_maps(inputs_seq, W_ih0, W_hh0, b0, W_hr0, W_ih1,
                            W_hh1, b1, W_hr1, h_init, c_init, tokens)
    import time as _time
    try:
        runner = _get_runner()
        dev_in = runner.put_inputs(in_maps)
        _CACHE["dev_in"] = dev_in
        _t0 = _time.perf_counter()
        out_arrs = runner.execute(dev_in, runner.put_outbufs())
        runner.jax.block_until_ready(out_arrs)
        _CACHE["exec_wall_ns"] = (_time.perf_counter() - _t0) * 1e9
        results = runner.fetch(out_arrs)
    except Exception:
        # fall back to the stock SPMD path
        nc = _get_program()
        _t0 = _time.perf_counter()
        res = run_bass_kernel_spmd(nc, in_maps, core_ids=list(range(NCORES)))
        _CACHE["exec_wall_ns"] = (_time.perf_counter() - _t0) * 1e9
        results = res.results
    return _unshard(results)

